# revision 29
# baseline (speedup 1.0000x reference)
"""MoE MLP (top-2 of 8 experts) Trainium2 kernel — expert-parallel across 8 NeuronCores.

Strategy (v2):
  - Router data-parallel: each core computes logits for its 512-token shard with
    float32r PE matmuls (f32-exact numerics, 1 cycle/row), AllGathers a tiny
    per-token record [e1, e2, w1, w2] (4096 x 4 fp32).
  - Every core replicates the position computation (compact-slot assignment via
    triangular-matrix prefix-sum matmuls on the PE).
  - Each core owns ONE expert. It compacts (token_id, token_id, gating) triples
    via ONE indirect-DMA scatter (OOB-skip for unassigned), gathers the assigned
    token rows (bf16) from its HBM copy of x, transposes them to d-major via
    DMA-transpose on the SP queue, runs x@W1 -> relu^2 -> @W2 in bf16 on the PE,
    scales rows by the gating weight, and indirect-scatters the weighted rows
    into a host-zeroed dense (N_TOK+1, D) bf16 buffer at their token positions
    (padding slots land in the trash row N_TOK).
  - Combine: ONE ReduceScatter(add) over the dense buffers writes each core's
    512-token fp32-accumulated output shard directly (collective cost is
    out-bytes-based: 1 MB vs the 18.9 MB an AllGather of compacts would move).
  - PE kept warm with junk matmuls through the router/AllGather gaps so the
    p-state ramp doesn't tax the main matmuls.
"""
import sys, os
sys.path.insert(0, "/opt/trn_rl_repo")
import numpy as np
import ml_dtypes

import concourse.bass as bass
import concourse.bacc as bacc
import concourse.mybir as mybir
from concourse.tile import TileContext
from concourse.bass import IndirectOffsetOnAxis

P = 128
N_TOK = 4096      # B*T
D = 1024
E = 8
H = 2048
R = 8             # cores = experts
SH = N_TOK // R   # 512 tokens per shard
G = N_TOK // P    # 32 global 128-token chunks
GSH = G // R      # 4 chunks per shard
C = 1152          # expert capacity (max observed load 1091; binomial mean 1024, sd 28)
CB = C // P       # 9 capacity blocks
BIG = float(1 << 20)
F32 = mybir.dt.float32
F32R = mybir.dt.float32r
BF16 = mybir.dt.bfloat16
I32 = mybir.dt.int32

GROUPS = [(0, 512), (512, 512), (1024, 128)]   # mm slot groups (offset, width)
SCATTER_BATCH = 1      # g-chunks per indirect scatter instruction (1 = safe loop)
STRIDE0_ZERO = True
DMA_TRANSPOSE = True
XT_CHUNKS = 4          # router operand load split for early matmul start


def build_kernel():
    nc = bacc.Bacc(None, dynamic_dma_scratch_size=32768)

    # ---------------- I/O ----------------
    xT_shard = nc.declare_dram_parameter("xT_shard", [D, SH], F32, isOutput=False)
    x_bf = nc.declare_dram_parameter("x_bf", [N_TOK, D], BF16, isOutput=False)
    w1_in = nc.declare_dram_parameter("w1", [D, H], BF16, isOutput=False)
    w2_in = nc.declare_dram_parameter("w2", [H, D], BF16, isOutput=False)
    wg_in = nc.declare_dram_parameter("wg", [D, E], F32, isOutput=False)
    # constants
    ident_in = nc.declare_dram_parameter("ident", [P, P], F32, isOutput=False)
    lstrict_in = nc.declare_dram_parameter("lstrict", [P, P], F32, isOutput=False)  # [k,m]=1 iff k<m
    le00_in = nc.declare_dram_parameter("le00", [P, P], F32, isOutput=False)  # [(g',e'),(g,e)] e'==e & g'<g
    le01_in = nc.declare_dram_parameter("le01", [P, P], F32, isOutput=False)  # e'==e (all)
    iota8_in = nc.declare_dram_parameter("iota8", [P, E], F32, isOutput=False)   # rows = 0..7
    iotat_in = nc.declare_dram_parameter("iotat", [P, G], F32, isOutput=False)   # [p,g] = 128g+p
    onehr_in = nc.declare_dram_parameter("onehr", [P, E], F32, isOutput=False)   # rows = onehot(core)
    out_shard = nc.declare_dram_parameter("out_shard", [SH, D], BF16, isOutput=True)

    # ---------------- internal DRAM ----------------
    rec_own_d = nc.dram_tensor("rec_own_d", [SH, 4], F32)
    rec_all_d = nc.dram_tensor("rec_all_d", [N_TOK, 4], F32, addr_space="Shared")
    # compact buffer, device-prefilled with (gather_id=0, scatter_id=N_TOK, gating=0)
    comp = nc.dram_tensor("comp", [C, 3], F32)
    # dense per-core output buffers (column halves), device-zeroed; row N_TOK is
    # the padding trash row
    DH = D // 2
    y_half = [nc.dram_tensor("y_half%d" % h, [N_TOK + 1, DH], BF16) for h in range(2)]
    zrow_d = nc.dram_tensor("zrow_d", [1, D], BF16)
    y_rs = [nc.dram_tensor("y_rs%d" % h, [SH, DH], BF16) for h in range(2)]

    with TileContext(nc) as tc:
        with tc.tile_pool(name="const", bufs=1) as cp, \
             tc.tile_pool(name="wpool", bufs=1) as wp, \
             tc.tile_pool(name="sb", bufs=2) as sb, \
             tc.tile_pool(name="big", bufs=1) as bigp, \
             tc.tile_pool(name="ps", bufs=1, space="PSUM") as ps, \
             tc.tile_pool(name="mmps", bufs=4, space="PSUM") as mmps:

            # ---- early loads, spread over the three DMA-capable queues ----
            # SP: ident (warm fodder) then the router operand in chunks.
            wg_sb = cp.tile([P, D // P, E], F32)
            nc.sync.dma_start(out=wg_sb[:], in_=wg_in.rearrange('(dc p) e -> p dc e', p=P))
            ident = cp.tile([P, P], F32)
            nc.sync.dma_start(out=ident[:], in_=ident_in[:])
            DCC = (D // P) // XT_CHUNKS      # dc per chunk
            xT_c = [bigp.tile([P, DCC, SH], F32, name="xT_c%d" % k) for k in range(XT_CHUNKS)]
            xTr = xT_shard.rearrange('(dc p) t -> p dc t', p=P)
            for k in range(XT_CHUNKS):
                nc.sync.dma_start(out=xT_c[k][:], in_=xTr[:, k * DCC:(k + 1) * DCC, :])
            # Act: activation-table preload.
            ones_atl = cp.tile([1, 1], F32)
            nc.vector.memset(ones_atl[:], 1.0)
            atl = cp.tile([1, 1], F32)
            nc.scalar.activation(out=atl[:], in_=ones_atl[:], func=mybir.ActivationFunctionType.Sigmoid)
            iota8 = cp.tile([P, E], F32)
            nc.gpsimd.dma_start(out=iota8[:], in_=iota8_in[:])
            # Pool: position-machinery constants.
            lstrict = cp.tile([P, P], F32)
            nc.gpsimd.dma_start(out=lstrict[:], in_=lstrict_in[:])
            le00 = cp.tile([P, P], F32)
            nc.gpsimd.dma_start(out=le00[:], in_=le00_in[:])
            le01 = cp.tile([P, P], F32)
            nc.gpsimd.dma_start(out=le01[:], in_=le01_in[:])
            iotat = cp.tile([P, G], F32)
            nc.gpsimd.dma_start(out=iotat[:], in_=iotat_in[:])
            onehr = cp.tile([P, E], F32)
            nc.gpsimd.dma_start(out=onehr[:], in_=onehr_in[:])
            ones_1p = cp.tile([1, P], F32)
            nc.vector.memset(ones_1p[:], 1.0)
            ones_col = cp.tile([P, 1], F32)
            nc.vector.memset(ones_col[:], 1.0)

            # ---- device-side init of comp prefill and the dense y buffer ----
            zb = cp.tile([P, D], BF16)
            nc.vector.memset(zb[:], 0.0)
            nc.scalar.dma_start(out=bass.AP(zrow_d, 0, [[D, 1], [1, D]]), in_=zb[0:1, :])
            for h in range(2):
                nc.scalar.dma_start(out=bass.AP(y_half[h], 0, [[DH, N_TOK + 1], [1, DH]]),
                                    in_=bass.AP(zrow_d, 0, [[0, N_TOK + 1], [1, DH]]))
            t3 = cp.tile([P, CB, 3], F32)
            nc.vector.memset(t3[:], 0.0)
            nc.vector.memset(t3[:, :, 1:2], float(N_TOK))
            nc.sync.dma_start(out=bass.AP(comp, 0, [[3, P], [3 * P, CB], [1, 3]]), in_=t3[:])

            # ---- PE warmup #1: keep the p-state ramp going until xT chunk 0 arrives ----
            warm_ps = ps.tile([P, 512], F32, space="PSUM", tag="warm")
            for _ in range(4):
                nc.tensor.matmul(out=warm_ps[:, 0:P], lhsT=ident[:], rhs=ident[:],
                                 start=True, stop=True, skip_group_check=True)

            # ---- router on own shard: f32 matmuls, chunk-pipelined with the load ----
            lgT_ps = ps.tile([E, SH], F32, space="PSUM", tag="pb")
            for k in range(XT_CHUNKS):
                for dck in range(DCC):
                    dc = k * DCC + dck
                    nc.tensor.matmul(out=lgT_ps[:], lhsT=wg_sb[:, dc, :], rhs=xT_c[k][:, dck, :],
                                     start=(dc == 0), stop=(dc == D // P - 1))
            lgT = sb.tile([E, SH], F32, tag="lgT")
            nc.vector.tensor_copy(out=lgT[:], in_=lgT_ps[:])
            logits = sb.tile([P, GSH, E], F32, tag="logits")
            for c in range(GSH):
                tp = ps.tile([P, E], F32, space="PSUM", tag="pc")
                nc.tensor.transpose(out=tp[:], in_=lgT[:, c * P:(c + 1) * P], identity=ident[:E, :E])
                nc.vector.tensor_copy(out=logits[:, c, :], in_=tp[:])

            mx = sb.tile([P, GSH, E], F32, tag="mx")
            for c in range(GSH):
                nc.vector.max(out=mx[:, c, :], in_=logits[:, c, :])
            m1 = mx[:, :, 0:1]
            m2 = mx[:, :, 1:2]
            dlt = sb.tile([P, GSH, 1], F32, tag="dlt")
            nc.vector.tensor_sub(out=dlt[:], in0=m1, in1=m2)
            rec_own = sb.tile([P, GSH, 4], F32, tag="rec_own")
            # w1 = sigmoid(m1-m2), w2 = sigmoid(m2-m1)
            nc.scalar.activation(out=rec_own[:, :, 2:3], in_=dlt[:], func=mybir.ActivationFunctionType.Sigmoid)
            nc.scalar.activation(out=rec_own[:, :, 3:4], in_=dlt[:], func=mybir.ActivationFunctionType.Sigmoid, scale=-1.0)
            # e1/e2 via onehot dot iota8
            oh = sb.tile([P, GSH, E], F32, tag="oh")
            tmp = sb.tile([P, GSH, E], F32, tag="ohtmp")
            nc.vector.tensor_tensor(out=oh[:], in0=logits[:], in1=m1.to_broadcast([P, GSH, E]),
                                    op=mybir.AluOpType.is_equal)
            nc.vector.tensor_tensor(out=tmp[:], in0=oh[:], in1=iota8[:].unsqueeze(1).to_broadcast([P, GSH, E]),
                                    op=mybir.AluOpType.mult)
            nc.vector.tensor_reduce(out=rec_own[:, :, 0:1], in_=tmp[:], axis=mybir.AxisListType.X,
                                    op=mybir.AluOpType.add)
            nc.vector.tensor_tensor(out=oh[:], in0=logits[:], in1=m2.to_broadcast([P, GSH, E]),
                                    op=mybir.AluOpType.is_equal)
            nc.vector.tensor_tensor(out=tmp[:], in0=oh[:], in1=iota8[:].unsqueeze(1).to_broadcast([P, GSH, E]),
                                    op=mybir.AluOpType.mult)
            nc.vector.tensor_reduce(out=rec_own[:, :, 1:2], in_=tmp[:], axis=mybir.AxisListType.X,
                                    op=mybir.AluOpType.add)
            # ship record on the Pool queue (SP is busy with w1): row t = 128c+p
            nc.gpsimd.dma_start(out=bass.AP(rec_own_d, 0, [[4, P], [SH, GSH], [1, 4]]), in_=rec_own[:])
            nc.gpsimd.collective_compute(
                "AllGather", mybir.AluOpType.bypass,
                ins=[rec_own_d[:]], outs=[rec_all_d[:]],
                replica_groups=[list(range(R))],
            )

            # w1 on SP right after the xT chunks; w2 on Act held past the record sigmoids.
            w1sb = wp.tile([P, D // P, H], BF16)   # [p, dc, h] = W1[dc*128+p, h]
            nc.sync.dma_start(out=w1sb[:], in_=w1_in.rearrange('(dc p) h -> p dc h', p=P))
            w2sb = wp.tile([P, H // P, D], BF16)   # [p, jj, d] = W2[jj*128+p, d]
            with tc.tile_wait_until(0.020):
                nc.scalar.dma_start(out=w2sb[:], in_=w2_in.rearrange('(jj p) d -> p jj d', p=P))

            # ---- PE warmup #2: cover the AllGather window ----
            with tc.tile_wait_until(0.0155):
                for i in range(24):
                    nc.tensor.matmul(out=warm_ps[0:E, :], lhsT=wg_sb[:, i % 8, :],
                                     rhs=xT_c[i % XT_CHUNKS][:, i % DCC, :],
                                     start=True, stop=True, skip_group_check=True)

            # ---- replicated positions over all tokens ----
            rec = sb.tile([P, G, 4], F32, tag="rec")
            nc.sync.dma_start(out=rec[:], in_=rec_all_d.rearrange('(g p) f -> p g f', p=P))
            e1a = rec[:, :, 0:1]
            e2a = rec[:, :, 1:2]
            w1a = rec[:, :, 2:3]
            w2a = rec[:, :, 3:4]
            oh1 = bigp.tile([P, G, E], F32)
            oh2 = bigp.tile([P, G, E], F32)
            i8b = iota8[:].unsqueeze(1).to_broadcast([P, G, E])
            nc.vector.tensor_tensor(out=oh1[:], in0=e1a.to_broadcast([P, G, E]), in1=i8b, op=mybir.AluOpType.is_equal)
            nc.vector.tensor_tensor(out=oh2[:], in0=e2a.to_broadcast([P, G, E]), in1=i8b, op=mybir.AluOpType.is_equal)
            mask = bigp.tile([P, G, E], F32)
            nc.vector.tensor_add(out=mask[:], in0=oh1[:], in1=oh2[:])
            mask2 = mask[:].rearrange('p g e -> p (g e)')

            pos_ps = ps.tile([P, G * E], F32, space="PSUM", tag="pe")
            nc.tensor.matmul(out=pos_ps[:], lhsT=lstrict[:], rhs=mask2, start=True, stop=False)
            # totals per (g,e), partition-major halves
            t0_ps = ps.tile([P, 1], F32, space="PSUM", tag="pb")
            nc.tensor.matmul(out=t0_ps[:], lhsT=mask2[:, 0:P], rhs=ones_col[:], start=True, stop=True)
            t1_ps = ps.tile([P, 1], F32, space="PSUM", tag="pc")
            nc.tensor.matmul(out=t1_ps[:], lhsT=mask2[:, P:2 * P], rhs=ones_col[:], start=True, stop=True)
            t0 = sb.tile([P, 1], F32, tag="t0sb")
            nc.vector.tensor_copy(out=t0[:], in_=t0_ps[:])
            t1 = sb.tile([P, 1], F32, tag="t1sb")
            nc.vector.tensor_copy(out=t1[:], in_=t1_ps[:])
            off0_ps = ps.tile([P, 1], F32, space="PSUM", tag="pb")
            nc.tensor.matmul(out=off0_ps[:], lhsT=le00[:], rhs=t0[:], start=True, stop=True)
            off1_ps = ps.tile([P, 1], F32, space="PSUM", tag="pc")
            nc.tensor.matmul(out=off1_ps[:], lhsT=le01[:], rhs=t0[:], start=True, stop=False)
            nc.tensor.matmul(out=off1_ps[:], lhsT=le00[:], rhs=t1[:], start=False, stop=True)
            off0 = sb.tile([P, 1], F32, tag="off0sb")
            nc.vector.tensor_copy(out=off0[:], in_=off0_ps[:])
            off1 = sb.tile([P, 1], F32, tag="off1sb")
            nc.vector.tensor_copy(out=off1[:], in_=off1_ps[:])
            offT_ps = ps.tile([1, P], F32, space="PSUM", tag="pb")
            offs_1p = sb.tile([1, 2 * P], F32, tag="offs1p")
            nc.tensor.transpose(out=offT_ps[:], in_=off0[:], identity=ident[:])
            nc.vector.tensor_copy(out=offs_1p[:, 0:P], in_=offT_ps[:])
            offT2_ps = ps.tile([1, P], F32, space="PSUM", tag="pc")
            nc.tensor.transpose(out=offT2_ps[:], in_=off1[:], identity=ident[:])
            nc.vector.tensor_copy(out=offs_1p[:, P:2 * P], in_=offT2_ps[:])
            # replicate chunk offsets to all partitions, accumulating into pos_ps
            nc.tensor.matmul(out=pos_ps[:], lhsT=ones_1p[:], rhs=offs_1p[:], start=False, stop=True)
            pos = bigp.tile([P, G, E], F32)
            nc.vector.tensor_copy(out=pos[:], in_=pos_ps[:].rearrange('p (g e) -> p g e', g=G))

            # ---- producer: gating + one-shot scatter compaction for own expert ----
            st = bigp.tile([P, G, E], F32)
            isr1 = sb.tile([P, G], F32, tag="isr1")
            isr2 = sb.tile([P, G], F32, tag="isr2")
            ohrb = onehr[:].unsqueeze(1).to_broadcast([P, G, E])
            nc.vector.tensor_tensor(out=st[:], in0=oh1[:], in1=ohrb, op=mybir.AluOpType.mult)
            nc.vector.tensor_reduce(out=isr1[:], in_=st[:], axis=mybir.AxisListType.X, op=mybir.AluOpType.add)
            nc.vector.tensor_tensor(out=st[:], in0=oh2[:], in1=ohrb, op=mybir.AluOpType.mult)
            nc.vector.tensor_reduce(out=isr2[:], in_=st[:], axis=mybir.AxisListType.X, op=mybir.AluOpType.add)
            g_r = sb.tile([P, G], F32, tag="g_r")
            tmpg2 = sb.tile([P, G], F32, tag="tmpg2")
            nc.vector.tensor_tensor(out=g_r[:], in0=isr1[:], in1=w1a.rearrange('p g o -> p (g o)'), op=mybir.AluOpType.mult)
            nc.vector.tensor_tensor(out=tmpg2[:], in0=isr2[:], in1=w2a.rearrange('p g o -> p (g o)'), op=mybir.AluOpType.mult)
            nc.vector.tensor_add(out=g_r[:], in0=g_r[:], in1=tmpg2[:])
            maskr = sb.tile([P, G], F32, tag="maskr")
            nc.vector.tensor_add(out=maskr[:], in0=isr1[:], in1=isr2[:])
            pos_r = sb.tile([P, G], F32, tag="pos_r")
            nc.vector.tensor_tensor(out=st[:], in0=mask[:], in1=ohrb, op=mybir.AluOpType.mult)
            nc.vector.tensor_tensor(out=st[:], in0=st[:], in1=pos[:], op=mybir.AluOpType.mult)
            nc.vector.tensor_reduce(out=pos_r[:], in_=st[:], axis=mybir.AxisListType.X, op=mybir.AluOpType.add)
            # scatter offsets: pos_r + BIG*(1-maskr)
            offsc = sb.tile([P, G], F32, tag="offsc")
            nc.vector.tensor_scalar_mul(tmpg2[:], maskr[:], -BIG)
            nc.vector.tensor_scalar_add(offsc[:], tmpg2[:], BIG)
            nc.vector.tensor_add(out=offsc[:], in0=offsc[:], in1=pos_r[:])
            offsc_i = sb.tile([P, G], I32, tag="offsci")
            nc.vector.tensor_copy(out=offsc_i[:], in_=offsc[:])
            vals = sb.tile([P, G, 3], F32, tag="vals")
            nc.vector.tensor_copy(out=vals[:, :, 0], in_=iotat[:])
            nc.vector.tensor_copy(out=vals[:, :, 1], in_=iotat[:])
            nc.vector.tensor_copy(out=vals[:, :, 2], in_=g_r[:])
            vals2 = vals[:].rearrange('p g f -> p (g f)')
            # scatter the (token, token, gating) triples, OOB-skip unassigned
            for g in range(0, G, SCATTER_BATCH):
                src = vals[:, g, :] if SCATTER_BATCH == 1 else vals2[:, g * 3:(g + SCATTER_BATCH) * 3]
                nc.gpsimd.indirect_dma_start(
                    out=comp[:],
                    out_offset=IndirectOffsetOnAxis(ap=offsc_i[:, g:g + SCATTER_BATCH], axis=0),
                    in_=src, in_offset=None,
                    bounds_check=C - 1, oob_is_err=False,
                )
            # reload compact ids & gatings (padding rows keep host prefill: 0 / N_TOK / 0)
            ids_f = sb.tile([P, CB], F32, tag="idsf")
            nc.scalar.dma_start(out=ids_f[:], in_=bass.AP(comp, 0, [[3, P], [3 * P, CB]]))
            idsc_f = sb.tile([P, CB], F32, tag="idscf")
            nc.sync.dma_start(out=idsc_f[:], in_=bass.AP(comp, 1, [[3, P], [3 * P, CB]]))
            g_load = sb.tile([P, CB], F32, tag="gload")
            nc.scalar.dma_start(out=g_load[:], in_=bass.AP(comp, 2, [[3, P], [3 * P, CB]]))
            ids_i = sb.tile([P, CB], I32, tag="idsi")
            nc.vector.tensor_copy(out=ids_i[:], in_=ids_f[:])
            idsc_i = sb.tile([P, CB], I32, tag="idsci")
            nc.vector.tensor_copy(out=idsc_i[:], in_=idsc_f[:])

            # ---- gather x rows (token-major), DMA-transpose to d-major per group ----
            xTg = [bigp.tile([P, D // P, n], BF16, name="xTg_%d" % gi)
                   for gi, (o, n) in enumerate(GROUPS)]
            for c in range(CB):
                gi = 0 if c < 4 else (1 if c < 8 else 2)
                o = GROUPS[gi][0]
                lc = c * P - o   # column offset within group tile
                xg_c = bigp.tile([P, D], BF16, tag="xgc", name="xg_%d" % c, bufs=6)
                nc.gpsimd.indirect_dma_start(
                    out=xg_c[:], out_offset=None,
                    in_=x_bf[:],
                    in_offset=IndirectOffsetOnAxis(ap=ids_i[:, c:c + 1], axis=0),
                )
                if DMA_TRANSPOSE:
                    nc.sync.dma_start_transpose(out=xTg[gi][:, :, lc:lc + P], in_=xg_c[:])
                else:
                    identb = cp.tile([P, P], BF16, name="identb")
                    if c == 0:
                        nc.vector.tensor_copy(out=identb[:], in_=ident[:])
                    for dc in range(D // P):
                        tps = ps.tile([P, P], BF16, space="PSUM", tag="rot", bufs=2)
                        nc.tensor.transpose(out=tps[:], in_=xg_c[:, dc * P:(dc + 1) * P], identity=identb[:])
                        nc.vector.tensor_copy(out=xTg[gi][:, dc, lc:lc + P], in_=tps[:])

            # ---- PE warmup #3: small junk matmuls bridge the scatter/gather window ----
            with tc.tile_wait_until(0.040):
                for i in range(80):
                    nc.tensor.matmul(out=warm_ps[:, 0:P], lhsT=ident[:], rhs=ident[:],
                                     start=True, stop=True, skip_group_check=True)

            # ---- mm1: hT[j] = relu(x W1)^2, h-major, per slot group ----
            hT = [bigp.tile([P, H // P, n], BF16, name="hT_%d" % gi)
                  for gi, (o, n) in enumerate(GROUPS)]
            for gi, (o, n) in enumerate(GROUPS):
                for j in range(H // P):
                    hps = mmps.tile([P, 512], F32, space="PSUM", tag="mm",
                                    name="hps_%d_%d" % (gi, j))
                    for dc in range(D // P):
                        nc.tensor.matmul(out=hps[:, :n], lhsT=w1sb[:, dc, j * P:(j + 1) * P],
                                         rhs=xTg[gi][:, dc, :],
                                         start=(dc == 0), stop=(dc == D // P - 1))
                    rl = sb.tile([P, 512], F32, tag="rl", name="rl_%d_%d" % (gi, j), bufs=4)
                    nc.scalar.activation(out=rl[:, :n], in_=hps[:, :n], func=mybir.ActivationFunctionType.Relu)
                    nc.vector.tensor_tensor(out=hT[gi][:, j, :], in0=rl[:, :n], in1=rl[:, :n],
                                            op=mybir.AluOpType.mult)

            # ---- mm2: y = hT^T W2, half-column passes so RS(half0) overlaps pass 1 ----
            for dn in range(2):
                for m in range(CB):
                    gi = 0 if m < 4 else (1 if m < 8 else 2)
                    o = GROUPS[gi][0]
                    lm = m * P - o
                    yrow = sb.tile([P, DH], BF16, tag="yrow", name="yrow_%d_%d" % (dn, m), bufs=12)
                    yps = mmps.tile([P, 512], F32, space="PSUM", tag="mm",
                                    name="yps_%d_%d" % (m, dn))
                    for jj in range(H // P):
                        nc.tensor.matmul(out=yps[:], lhsT=hT[gi][:, jj, lm:lm + P],
                                         rhs=w2sb[:, jj, dn * DH:(dn + 1) * DH],
                                         start=(jj == 0), stop=(jj == H // P - 1))
                    nc.scalar.activation(out=yrow[:], in_=yps[:],
                                         func=mybir.ActivationFunctionType.Copy,
                                         scale=g_load[:, m:m + 1])
                    nc.gpsimd.indirect_dma_start(
                        out=y_half[dn][:],
                        out_offset=IndirectOffsetOnAxis(ap=idsc_i[:, m:m + 1], axis=0),
                        in_=yrow[:], in_offset=None,
                    )
                # combine this half: ReduceScatter(add) overlaps the next pass
                nc.gpsimd.collective_compute(
                    "ReduceScatter", mybir.AluOpType.add,
                    ins=[y_half[dn][0:N_TOK, :]], outs=[y_rs[dn][:]],
                    replica_groups=[list(range(R))],
                )
                eng = nc.sync if dn == 0 else nc.scalar
                eng.dma_start(out=bass.AP(out_shard, dn * DH, [[D, SH], [1, DH]]),
                              in_=bass.AP(y_rs[dn], 0, [[DH, SH], [1, DH]]))

    nc.finalize()
    return nc


# ---------------- host-side constants ----------------
def host_constants():
    ident = np.eye(P, dtype=np.float32)
    lstrict = np.triu(np.ones((P, P), np.float32), k=1)  # [k, m] = 1 iff m > k
    # rows/cols indexed by (g*8 + e) within a 128-slot half (16 g values)
    gg, ee = np.arange(16), np.arange(E)
    gi = np.repeat(gg, E)   # g of row index
    ei = np.tile(ee, 16)    # e of row index
    le00 = ((ei[:, None] == ei[None, :]) & (gi[:, None] < gi[None, :])).astype(np.float32)
    le01 = (ei[:, None] == ei[None, :]).astype(np.float32)
    iota8 = np.broadcast_to(np.arange(E, dtype=np.float32), (P, E)).copy()
    iotat = (np.arange(G, dtype=np.float32)[None, :] * P + np.arange(P, dtype=np.float32)[:, None]).copy()
    return ident, lstrict, le00, le01, iota8, iotat


def build_in_maps(x, Wg, W1, W2):
    x = np.asarray(x); Wg = np.asarray(Wg); W1 = np.asarray(W1); W2 = np.asarray(W2)
    xt = x.reshape(N_TOK, D).astype(np.float32)
    x_bf = xt.astype(ml_dtypes.bfloat16)
    ident, lstrict, le00, le01, iota8, iotat = host_constants()
    in_maps = []
    for r in range(R):
        onehr = np.zeros((P, E), np.float32); onehr[:, r] = 1.0
        in_maps.append({
            "xT_shard": np.ascontiguousarray(xt[r * SH:(r + 1) * SH, :].T),
            "x_bf": x_bf,
            "w1": W1[r].astype(ml_dtypes.bfloat16),
            "w2": W2[r].astype(ml_dtypes.bfloat16),
            "wg": Wg.astype(np.float32),
            "ident": ident, "lstrict": lstrict, "le00": le00, "le01": le01,
            "iota8": iota8, "iotat": iotat, "onehr": onehr,
        })
    return in_maps


_NC_CACHE = {}

def kernel(x, Wg, W1, W2):
    x = np.asarray(x)
    B, T, Dx = x.shape
    in_maps = build_in_maps(x, Wg, W1, W2)
    if "nc" not in _NC_CACHE:
        _NC_CACHE["nc"] = build_kernel()
    from concourse.bass_utils import run_bass_kernel_spmd
    res = run_bass_kernel_spmd(_NC_CACHE["nc"], in_maps, list(range(R)))
    globals()['LAST_RES'] = res
    out = np.concatenate([np.asarray(res.results[r]["out_shard"]).astype(np.float32)
                          for r in range(R)], axis=0)
    return out.reshape(B, T, Dx)


if __name__ == "__main__":
    d = np.load("/tmp/inputs.npz")
    out = kernel(d["x"], d["Wg"], d["W1"], d["W2"])
    ref = np.load("/tmp/ref_out.npy")
    err = np.abs(out - ref).max() / np.abs(ref).max()
    print("rel err (absmax):", err)


# revision 31
# speedup vs baseline: 1.0609x; 1.0609x over previous
"""MoE MLP (top-2 of 8 experts) Trainium2 kernel — expert-parallel across 8 NeuronCores.

Strategy (v2):
  - Router data-parallel: each core computes logits for its 512-token shard with
    float32r PE matmuls (f32-exact numerics, 1 cycle/row), AllGathers a tiny
    per-token record [e1, e2, w1, w2] (4096 x 4 fp32).
  - Every core replicates the position computation (compact-slot assignment via
    triangular-matrix prefix-sum matmuls on the PE).
  - Each core owns ONE expert. It compacts (token_id, token_id, gating) triples
    via ONE indirect-DMA scatter (OOB-skip for unassigned), gathers the assigned
    token rows (bf16) from its HBM copy of x, transposes them to d-major via
    DMA-transpose on the SP queue, runs x@W1 -> relu^2 -> @W2 in bf16 on the PE,
    scales rows by the gating weight, and indirect-scatters the weighted rows
    into a host-zeroed dense (N_TOK+1, D) bf16 buffer at their token positions
    (padding slots land in the trash row N_TOK).
  - Combine: ONE ReduceScatter(add) over the dense buffers writes each core's
    512-token fp32-accumulated output shard directly (collective cost is
    out-bytes-based: 1 MB vs the 18.9 MB an AllGather of compacts would move).
  - PE kept warm with junk matmuls through the router/AllGather gaps so the
    p-state ramp doesn't tax the main matmuls.
"""
import sys, os
sys.path.insert(0, "/opt/trn_rl_repo")
import numpy as np
import ml_dtypes

import concourse.bass as bass
import concourse.bacc as bacc
import concourse.mybir as mybir
from concourse.tile import TileContext
from concourse.bass import IndirectOffsetOnAxis

P = 128
N_TOK = 4096      # B*T
D = 1024
E = 8
H = 2048
R = 8             # cores = experts
SH = N_TOK // R   # 512 tokens per shard
G = N_TOK // P    # 32 global 128-token chunks
GSH = G // R      # 4 chunks per shard
C = 1152          # expert capacity (max observed load 1091; binomial mean 1024, sd 28)
CB = C // P       # 9 capacity blocks
BIG = float(1 << 20)
F32 = mybir.dt.float32
F32R = mybir.dt.float32r
BF16 = mybir.dt.bfloat16
I32 = mybir.dt.int32

GROUPS = [(0, 512), (512, 512), (1024, 128)]   # mm slot groups (offset, width)
SCATTER_BATCH = 1      # g-chunks per indirect scatter instruction (1 = safe loop)
STRIDE0_ZERO = True
DMA_TRANSPOSE = True
XT_CHUNKS = 4          # router operand load split for early matmul start


def build_kernel():
    nc = bacc.Bacc(None, dynamic_dma_scratch_size=32768)

    # ---------------- I/O ----------------
    xT_shard = nc.declare_dram_parameter("xT_shard", [D, SH], F32, isOutput=False)
    x_bf = nc.declare_dram_parameter("x_bf", [N_TOK, D], BF16, isOutput=False)
    w1_in = nc.declare_dram_parameter("w1", [D, H], BF16, isOutput=False)
    w2_in = nc.declare_dram_parameter("w2", [H, D], BF16, isOutput=False)
    wg_in = nc.declare_dram_parameter("wg", [D, E], F32, isOutput=False)
    # constants
    ident_in = nc.declare_dram_parameter("ident", [P, P], F32, isOutput=False)
    lstrict_in = nc.declare_dram_parameter("lstrict", [P, P], F32, isOutput=False)  # [k,m]=1 iff k<m
    le00_in = nc.declare_dram_parameter("le00", [P, P], F32, isOutput=False)  # [(g',e'),(g,e)] e'==e & g'<g
    le01_in = nc.declare_dram_parameter("le01", [P, P], F32, isOutput=False)  # e'==e (all)
    iota8_in = nc.declare_dram_parameter("iota8", [P, E], F32, isOutput=False)   # rows = 0..7
    iotat_in = nc.declare_dram_parameter("iotat", [P, G], F32, isOutput=False)   # [p,g] = 128g+p
    onehr_in = nc.declare_dram_parameter("onehr", [P, E], F32, isOutput=False)   # rows = onehot(core)
    out_shard = nc.declare_dram_parameter("out_shard", [SH, D], BF16, isOutput=True)

    # ---------------- internal DRAM ----------------
    rec_own_d = nc.dram_tensor("rec_own_d", [SH, 4], F32)
    rec_all_d = nc.dram_tensor("rec_all_d", [N_TOK, 4], F32, addr_space="Shared")
    # compact buffer, device-prefilled with (gather_id=0, scatter_id=N_TOK, gating=0)
    comp = nc.dram_tensor("comp", [C, 3], F32)
    # dense per-core output buffers (column halves), device-zeroed; row N_TOK is
    # the padding trash row
    DH = D // 2
    y_half = [nc.dram_tensor("y_half%d" % h, [N_TOK + 1, DH], BF16) for h in range(2)]
    zrow_d = nc.dram_tensor("zrow_d", [1, D], BF16)
    y_rs = [nc.dram_tensor("y_rs%d" % h, [SH, DH], BF16) for h in range(2)]

    with TileContext(nc) as tc:
        with tc.tile_pool(name="const", bufs=1) as cp, \
             tc.tile_pool(name="wpool", bufs=1) as wp, \
             tc.tile_pool(name="sb", bufs=2) as sb, \
             tc.tile_pool(name="big", bufs=1) as bigp, \
             tc.tile_pool(name="ps", bufs=1, space="PSUM") as ps, \
             tc.tile_pool(name="mmps", bufs=3, space="PSUM") as mmps:

            # ---- early loads, spread over the three DMA-capable queues ----
            # SP: ident (warm fodder) then the router operand in chunks.
            wg_sb = cp.tile([P, D // P, E], F32)
            nc.sync.dma_start(out=wg_sb[:], in_=wg_in.rearrange('(dc p) e -> p dc e', p=P))
            ident = cp.tile([P, P], F32)
            nc.sync.dma_start(out=ident[:], in_=ident_in[:])
            DCC = (D // P) // XT_CHUNKS      # dc per chunk
            xT_c = [bigp.tile([P, DCC, SH], F32, name="xT_c%d" % k) for k in range(XT_CHUNKS)]
            xTr = xT_shard.rearrange('(dc p) t -> p dc t', p=P)
            for k in range(XT_CHUNKS):
                nc.sync.dma_start(out=xT_c[k][:], in_=xTr[:, k * DCC:(k + 1) * DCC, :])
            # Act: activation-table preload.
            ones_atl = cp.tile([1, 1], F32)
            nc.vector.memset(ones_atl[:], 1.0)
            atl = cp.tile([1, 1], F32)
            nc.scalar.activation(out=atl[:], in_=ones_atl[:], func=mybir.ActivationFunctionType.Sigmoid)
            iota8 = cp.tile([P, E], F32)
            nc.gpsimd.dma_start(out=iota8[:], in_=iota8_in[:])
            # Pool: position-machinery constants.
            lstrict = cp.tile([P, P], F32)
            nc.gpsimd.dma_start(out=lstrict[:], in_=lstrict_in[:])
            le00 = cp.tile([P, P], F32)
            nc.gpsimd.dma_start(out=le00[:], in_=le00_in[:])
            le01 = cp.tile([P, P], F32)
            nc.gpsimd.dma_start(out=le01[:], in_=le01_in[:])
            iotat = cp.tile([P, G], F32)
            nc.gpsimd.dma_start(out=iotat[:], in_=iotat_in[:])
            onehr = cp.tile([P, E], F32)
            nc.gpsimd.dma_start(out=onehr[:], in_=onehr_in[:])
            ones_1p = cp.tile([1, P], F32)
            nc.vector.memset(ones_1p[:], 1.0)
            ones_col = cp.tile([P, 1], F32)
            nc.vector.memset(ones_col[:], 1.0)
            identb = cp.tile([P, P], BF16)
            nc.vector.tensor_copy(out=identb[:], in_=ident[:])

            # ---- device-side init of comp prefill and the dense y buffer ----
            zb = cp.tile([P, D], BF16)
            nc.vector.memset(zb[:], 0.0)
            nc.scalar.dma_start(out=bass.AP(zrow_d, 0, [[D, 1], [1, D]]), in_=zb[0:1, :])
            for h in range(2):
                nc.scalar.dma_start(out=bass.AP(y_half[h], 0, [[DH, N_TOK + 1], [1, DH]]),
                                    in_=bass.AP(zrow_d, 0, [[0, N_TOK + 1], [1, DH]]))
            t3 = cp.tile([P, CB, 3], F32)
            nc.vector.memset(t3[:], 0.0)
            nc.vector.memset(t3[:, :, 1:2], float(N_TOK))
            nc.sync.dma_start(out=bass.AP(comp, 0, [[3, P], [3 * P, CB], [1, 3]]), in_=t3[:])

            # ---- PE warmup #1: keep the p-state ramp going until xT chunk 0 arrives ----
            warm_ps = ps.tile([P, 512], F32, space="PSUM", tag="warm")
            for _ in range(4):
                nc.tensor.matmul(out=warm_ps[:, 0:P], lhsT=ident[:], rhs=ident[:],
                                 start=True, stop=True, skip_group_check=True)

            # ---- router on own shard: f32 matmuls, chunk-pipelined with the load ----
            lgT_ps = ps.tile([E, SH], F32, space="PSUM", tag="pb")
            for k in range(XT_CHUNKS):
                for dck in range(DCC):
                    dc = k * DCC + dck
                    nc.tensor.matmul(out=lgT_ps[:], lhsT=wg_sb[:, dc, :], rhs=xT_c[k][:, dck, :],
                                     start=(dc == 0), stop=(dc == D // P - 1))
            lgT = sb.tile([E, SH], F32, tag="lgT")
            nc.vector.tensor_copy(out=lgT[:], in_=lgT_ps[:])
            logits = sb.tile([P, GSH, E], F32, tag="logits")
            for c in range(GSH):
                tp = ps.tile([P, E], F32, space="PSUM", tag="pb")
                nc.tensor.transpose(out=tp[:], in_=lgT[:, c * P:(c + 1) * P], identity=ident[:E, :E])
                nc.vector.tensor_copy(out=logits[:, c, :], in_=tp[:])

            mx = sb.tile([P, GSH, E], F32, tag="mx")
            for c in range(GSH):
                nc.vector.max(out=mx[:, c, :], in_=logits[:, c, :])
            m1 = mx[:, :, 0:1]
            m2 = mx[:, :, 1:2]
            dlt = sb.tile([P, GSH, 1], F32, tag="dlt")
            nc.vector.tensor_sub(out=dlt[:], in0=m1, in1=m2)
            rec_own = sb.tile([P, GSH, 4], F32, tag="rec_own")
            # w1 = sigmoid(m1-m2), w2 = sigmoid(m2-m1)
            nc.scalar.activation(out=rec_own[:, :, 2:3], in_=dlt[:], func=mybir.ActivationFunctionType.Sigmoid)
            nc.scalar.activation(out=rec_own[:, :, 3:4], in_=dlt[:], func=mybir.ActivationFunctionType.Sigmoid, scale=-1.0)
            # e1/e2 via onehot dot iota8
            oh = sb.tile([P, GSH, E], F32, tag="oh")
            tmp = sb.tile([P, GSH, E], F32, tag="ohtmp")
            nc.vector.tensor_tensor(out=oh[:], in0=logits[:], in1=m1.to_broadcast([P, GSH, E]),
                                    op=mybir.AluOpType.is_equal)
            nc.vector.tensor_tensor(out=tmp[:], in0=oh[:], in1=iota8[:].unsqueeze(1).to_broadcast([P, GSH, E]),
                                    op=mybir.AluOpType.mult)
            nc.vector.tensor_reduce(out=rec_own[:, :, 0:1], in_=tmp[:], axis=mybir.AxisListType.X,
                                    op=mybir.AluOpType.add)
            nc.vector.tensor_tensor(out=oh[:], in0=logits[:], in1=m2.to_broadcast([P, GSH, E]),
                                    op=mybir.AluOpType.is_equal)
            nc.vector.tensor_tensor(out=tmp[:], in0=oh[:], in1=iota8[:].unsqueeze(1).to_broadcast([P, GSH, E]),
                                    op=mybir.AluOpType.mult)
            nc.vector.tensor_reduce(out=rec_own[:, :, 1:2], in_=tmp[:], axis=mybir.AxisListType.X,
                                    op=mybir.AluOpType.add)
            # ship record on the Pool queue (SP is busy with w1): row t = 128c+p
            nc.gpsimd.dma_start(out=bass.AP(rec_own_d, 0, [[4, P], [SH, GSH], [1, 4]]), in_=rec_own[:])
            nc.gpsimd.collective_compute(
                "AllGather", mybir.AluOpType.bypass,
                ins=[rec_own_d[:]], outs=[rec_all_d[:]],
                replica_groups=[list(range(R))],
            )

            # w1 on SP right after the xT chunks; w2 on Act held past the record sigmoids.
            w1sb = wp.tile([P, D // P, H], BF16)   # [p, dc, h] = W1[dc*128+p, h]
            nc.sync.dma_start(out=w1sb[:], in_=w1_in.rearrange('(dc p) h -> p dc h', p=P))
            w2sb = wp.tile([P, H // P, D], BF16)   # [p, jj, d] = W2[jj*128+p, d]
            with tc.tile_wait_until(0.020):
                nc.scalar.dma_start(out=w2sb[:], in_=w2_in.rearrange('(jj p) d -> p jj d', p=P))

            # ---- PE warmup #2: cover the AllGather window ----
            with tc.tile_wait_until(0.0155):
                for i in range(24):
                    nc.tensor.matmul(out=warm_ps[0:E, :], lhsT=wg_sb[:, i % 8, :],
                                     rhs=xT_c[i % XT_CHUNKS][:, i % DCC, :],
                                     start=True, stop=True, skip_group_check=True)

            # ---- replicated positions over all tokens ----
            rec = sb.tile([P, G, 4], F32, tag="rec")
            nc.sync.dma_start(out=rec[:], in_=rec_all_d.rearrange('(g p) f -> p g f', p=P))
            e1a = rec[:, :, 0:1]
            e2a = rec[:, :, 1:2]
            w1a = rec[:, :, 2:3]
            w2a = rec[:, :, 3:4]
            oh1 = bigp.tile([P, G, E], F32)
            oh2 = bigp.tile([P, G, E], F32)
            i8b = iota8[:].unsqueeze(1).to_broadcast([P, G, E])
            nc.vector.tensor_tensor(out=oh1[:], in0=e1a.to_broadcast([P, G, E]), in1=i8b, op=mybir.AluOpType.is_equal)
            nc.vector.tensor_tensor(out=oh2[:], in0=e2a.to_broadcast([P, G, E]), in1=i8b, op=mybir.AluOpType.is_equal)
            mask = bigp.tile([P, G, E], F32)
            nc.vector.tensor_add(out=mask[:], in0=oh1[:], in1=oh2[:])
            mask2 = mask[:].rearrange('p g e -> p (g e)')

            pos_ps = ps.tile([P, G * E], F32, space="PSUM", tag="pe")
            nc.tensor.matmul(out=pos_ps[:], lhsT=lstrict[:], rhs=mask2, start=True, stop=False)
            # totals per (g,e), partition-major halves
            t0_ps = ps.tile([P, 1], F32, space="PSUM", tag="pb")
            nc.tensor.matmul(out=t0_ps[:], lhsT=mask2[:, 0:P], rhs=ones_col[:], start=True, stop=True)
            t1_ps = ps.tile([P, 1], F32, space="PSUM", tag="pb")
            nc.tensor.matmul(out=t1_ps[:], lhsT=mask2[:, P:2 * P], rhs=ones_col[:], start=True, stop=True)
            t0 = sb.tile([P, 1], F32, tag="t0sb")
            nc.vector.tensor_copy(out=t0[:], in_=t0_ps[:])
            t1 = sb.tile([P, 1], F32, tag="t1sb")
            nc.vector.tensor_copy(out=t1[:], in_=t1_ps[:])
            off0_ps = ps.tile([P, 1], F32, space="PSUM", tag="pb")
            nc.tensor.matmul(out=off0_ps[:], lhsT=le00[:], rhs=t0[:], start=True, stop=True)
            off1_ps = ps.tile([P, 1], F32, space="PSUM", tag="pb")
            nc.tensor.matmul(out=off1_ps[:], lhsT=le01[:], rhs=t0[:], start=True, stop=False)
            nc.tensor.matmul(out=off1_ps[:], lhsT=le00[:], rhs=t1[:], start=False, stop=True)
            off0 = sb.tile([P, 1], F32, tag="off0sb")
            nc.vector.tensor_copy(out=off0[:], in_=off0_ps[:])
            off1 = sb.tile([P, 1], F32, tag="off1sb")
            nc.vector.tensor_copy(out=off1[:], in_=off1_ps[:])
            offT_ps = ps.tile([1, P], F32, space="PSUM", tag="pb")
            offs_1p = sb.tile([1, 2 * P], F32, tag="offs1p")
            nc.tensor.transpose(out=offT_ps[:], in_=off0[:], identity=ident[:])
            nc.vector.tensor_copy(out=offs_1p[:, 0:P], in_=offT_ps[:])
            offT2_ps = ps.tile([1, P], F32, space="PSUM", tag="pb")
            nc.tensor.transpose(out=offT2_ps[:], in_=off1[:], identity=ident[:])
            nc.vector.tensor_copy(out=offs_1p[:, P:2 * P], in_=offT2_ps[:])
            # replicate chunk offsets to all partitions, accumulating into pos_ps
            nc.tensor.matmul(out=pos_ps[:], lhsT=ones_1p[:], rhs=offs_1p[:], start=False, stop=True)
            pos = bigp.tile([P, G, E], F32)
            nc.vector.tensor_copy(out=pos[:], in_=pos_ps[:].rearrange('p (g e) -> p g e', g=G))

            # ---- producer: gating + one-shot scatter compaction for own expert ----
            st = bigp.tile([P, G, E], F32)
            isr1 = sb.tile([P, G], F32, tag="isr1")
            isr2 = sb.tile([P, G], F32, tag="isr2")
            ohrb = onehr[:].unsqueeze(1).to_broadcast([P, G, E])
            nc.vector.tensor_tensor(out=st[:], in0=oh1[:], in1=ohrb, op=mybir.AluOpType.mult)
            nc.vector.tensor_reduce(out=isr1[:], in_=st[:], axis=mybir.AxisListType.X, op=mybir.AluOpType.add)
            nc.vector.tensor_tensor(out=st[:], in0=oh2[:], in1=ohrb, op=mybir.AluOpType.mult)
            nc.vector.tensor_reduce(out=isr2[:], in_=st[:], axis=mybir.AxisListType.X, op=mybir.AluOpType.add)
            g_r = sb.tile([P, G], F32, tag="g_r")
            tmpg2 = sb.tile([P, G], F32, tag="tmpg2")
            nc.vector.tensor_tensor(out=g_r[:], in0=isr1[:], in1=w1a.rearrange('p g o -> p (g o)'), op=mybir.AluOpType.mult)
            nc.vector.tensor_tensor(out=tmpg2[:], in0=isr2[:], in1=w2a.rearrange('p g o -> p (g o)'), op=mybir.AluOpType.mult)
            nc.vector.tensor_add(out=g_r[:], in0=g_r[:], in1=tmpg2[:])
            maskr = sb.tile([P, G], F32, tag="maskr")
            nc.vector.tensor_add(out=maskr[:], in0=isr1[:], in1=isr2[:])
            pos_r = sb.tile([P, G], F32, tag="pos_r")
            nc.vector.tensor_tensor(out=st[:], in0=mask[:], in1=ohrb, op=mybir.AluOpType.mult)
            nc.vector.tensor_tensor(out=st[:], in0=st[:], in1=pos[:], op=mybir.AluOpType.mult)
            nc.vector.tensor_reduce(out=pos_r[:], in_=st[:], axis=mybir.AxisListType.X, op=mybir.AluOpType.add)
            # scatter offsets: pos_r + BIG*(1-maskr)
            offsc = sb.tile([P, G], F32, tag="offsc")
            nc.vector.tensor_scalar_mul(tmpg2[:], maskr[:], -BIG)
            nc.vector.tensor_scalar_add(offsc[:], tmpg2[:], BIG)
            nc.vector.tensor_add(out=offsc[:], in0=offsc[:], in1=pos_r[:])
            offsc_i = sb.tile([P, G], I32, tag="offsci")
            nc.vector.tensor_copy(out=offsc_i[:], in_=offsc[:])
            vals = sb.tile([P, G, 3], F32, tag="vals")
            nc.vector.tensor_copy(out=vals[:, :, 0], in_=iotat[:])
            nc.vector.tensor_copy(out=vals[:, :, 1], in_=iotat[:])
            nc.vector.tensor_copy(out=vals[:, :, 2], in_=g_r[:])
            vals2 = vals[:].rearrange('p g f -> p (g f)')
            # scatter the (token, token, gating) triples, OOB-skip unassigned
            for g in range(0, G, SCATTER_BATCH):
                src = vals[:, g, :] if SCATTER_BATCH == 1 else vals2[:, g * 3:(g + SCATTER_BATCH) * 3]
                nc.gpsimd.indirect_dma_start(
                    out=comp[:],
                    out_offset=IndirectOffsetOnAxis(ap=offsc_i[:, g:g + SCATTER_BATCH], axis=0),
                    in_=src, in_offset=None,
                    bounds_check=C - 1, oob_is_err=False,
                )
            # reload compact ids & gatings (padding rows keep host prefill: 0 / N_TOK / 0)
            ids_f = sb.tile([P, CB], F32, tag="idsf")
            nc.scalar.dma_start(out=ids_f[:], in_=bass.AP(comp, 0, [[3, P], [3 * P, CB]]))
            idsc_f = sb.tile([P, CB], F32, tag="idscf")
            nc.sync.dma_start(out=idsc_f[:], in_=bass.AP(comp, 1, [[3, P], [3 * P, CB]]))
            g_load = sb.tile([P, CB], F32, tag="gload")
            nc.scalar.dma_start(out=g_load[:], in_=bass.AP(comp, 2, [[3, P], [3 * P, CB]]))
            ids_i = sb.tile([P, CB], I32, tag="idsi")
            nc.vector.tensor_copy(out=ids_i[:], in_=ids_f[:])
            idsc_i = sb.tile([P, CB], I32, tag="idsci")
            nc.vector.tensor_copy(out=idsc_i[:], in_=idsc_f[:])

            # ---- gather x rows (token-major), DMA-transpose to d-major per group ----
            xTg = [bigp.tile([P, D // P, n], BF16, name="xTg_%d" % gi)
                   for gi, (o, n) in enumerate(GROUPS)]
            for c in range(CB):
                gi = 0 if c < 4 else (1 if c < 8 else 2)
                o = GROUPS[gi][0]
                lc = c * P - o   # column offset within group tile
                xg_c = bigp.tile([P, D], BF16, tag="xgc", name="xg_%d" % c, bufs=6)
                nc.gpsimd.indirect_dma_start(
                    out=xg_c[:], out_offset=None,
                    in_=x_bf[:],
                    in_offset=IndirectOffsetOnAxis(ap=ids_i[:, c:c + 1], axis=0),
                )
                for dc in range(D // P):
                    tps = ps.tile([P, P], BF16, space="PSUM", tag="rot", bufs=2,
                                  name="tps_%d_%d" % (c, dc))
                    nc.tensor.transpose(out=tps[:], in_=xg_c[:, dc * P:(dc + 1) * P], identity=identb[:])
                    nc.vector.tensor_copy(out=xTg[gi][:, dc, lc:lc + P], in_=tps[:])


            # ---- mm1: hT[j] = relu(x W1)^2, h-major, per slot group ----
            hT = [bigp.tile([P, H // P, n], BF16, name="hT_%d" % gi)
                  for gi, (o, n) in enumerate(GROUPS)]
            for gi, (o, n) in enumerate(GROUPS):
                for j in range(H // P):
                    hps = mmps.tile([P, 512], F32, space="PSUM", tag="mm",
                                    name="hps_%d_%d" % (gi, j))
                    for dc in range(D // P):
                        nc.tensor.matmul(out=hps[:, :n], lhsT=w1sb[:, dc, j * P:(j + 1) * P],
                                         rhs=xTg[gi][:, dc, :],
                                         start=(dc == 0), stop=(dc == D // P - 1))
                    rl = sb.tile([P, 512], F32, tag="rl", name="rl_%d_%d" % (gi, j), bufs=4)
                    nc.scalar.activation(out=rl[:, :n], in_=hps[:, :n], func=mybir.ActivationFunctionType.Relu)
                    nc.vector.tensor_tensor(out=hT[gi][:, j, :], in0=rl[:, :n], in1=rl[:, :n],
                                            op=mybir.AluOpType.mult)

            # ---- mm2: y = hT^T W2, half-column passes so RS(half0) overlaps pass 1 ----
            for dn in range(2):
                for m in range(CB):
                    gi = 0 if m < 4 else (1 if m < 8 else 2)
                    o = GROUPS[gi][0]
                    lm = m * P - o
                    yrow = sb.tile([P, DH], BF16, tag="yrow", name="yrow_%d_%d" % (dn, m), bufs=12)
                    yps = mmps.tile([P, 512], F32, space="PSUM", tag="mm",
                                    name="yps_%d_%d" % (m, dn))
                    for jj in range(H // P):
                        nc.tensor.matmul(out=yps[:], lhsT=hT[gi][:, jj, lm:lm + P],
                                         rhs=w2sb[:, jj, dn * DH:(dn + 1) * DH],
                                         start=(jj == 0), stop=(jj == H // P - 1))
                    nc.scalar.activation(out=yrow[:], in_=yps[:],
                                         func=mybir.ActivationFunctionType.Copy,
                                         scale=g_load[:, m:m + 1])
                    nc.gpsimd.indirect_dma_start(
                        out=y_half[dn][:],
                        out_offset=IndirectOffsetOnAxis(ap=idsc_i[:, m:m + 1], axis=0),
                        in_=yrow[:], in_offset=None,
                    )
                # combine this half: ReduceScatter(add) overlaps the next pass
                nc.gpsimd.collective_compute(
                    "ReduceScatter", mybir.AluOpType.add,
                    ins=[y_half[dn][0:N_TOK, :]], outs=[y_rs[dn][:]],
                    replica_groups=[list(range(R))],
                )
                eng = nc.sync if dn == 0 else nc.scalar
                eng.dma_start(out=bass.AP(out_shard, dn * DH, [[D, SH], [1, DH]]),
                              in_=bass.AP(y_rs[dn], 0, [[DH, SH], [1, DH]]))

    nc.finalize()
    return nc


# ---------------- host-side constants ----------------
def host_constants():
    ident = np.eye(P, dtype=np.float32)
    lstrict = np.triu(np.ones((P, P), np.float32), k=1)  # [k, m] = 1 iff m > k
    # rows/cols indexed by (g*8 + e) within a 128-slot half (16 g values)
    gg, ee = np.arange(16), np.arange(E)
    gi = np.repeat(gg, E)   # g of row index
    ei = np.tile(ee, 16)    # e of row index
    le00 = ((ei[:, None] == ei[None, :]) & (gi[:, None] < gi[None, :])).astype(np.float32)
    le01 = (ei[:, None] == ei[None, :]).astype(np.float32)
    iota8 = np.broadcast_to(np.arange(E, dtype=np.float32), (P, E)).copy()
    iotat = (np.arange(G, dtype=np.float32)[None, :] * P + np.arange(P, dtype=np.float32)[:, None]).copy()
    return ident, lstrict, le00, le01, iota8, iotat


def build_in_maps(x, Wg, W1, W2):
    x = np.asarray(x); Wg = np.asarray(Wg); W1 = np.asarray(W1); W2 = np.asarray(W2)
    xt = x.reshape(N_TOK, D).astype(np.float32)
    x_bf = xt.astype(ml_dtypes.bfloat16)
    ident, lstrict, le00, le01, iota8, iotat = host_constants()
    in_maps = []
    for r in range(R):
        onehr = np.zeros((P, E), np.float32); onehr[:, r] = 1.0
        in_maps.append({
            "xT_shard": np.ascontiguousarray(xt[r * SH:(r + 1) * SH, :].T),
            "x_bf": x_bf,
            "w1": W1[r].astype(ml_dtypes.bfloat16),
            "w2": W2[r].astype(ml_dtypes.bfloat16),
            "wg": Wg.astype(np.float32),
            "ident": ident, "lstrict": lstrict, "le00": le00, "le01": le01,
            "iota8": iota8, "iotat": iotat, "onehr": onehr,
        })
    return in_maps


_NC_CACHE = {}

def kernel(x, Wg, W1, W2):
    x = np.asarray(x)
    B, T, Dx = x.shape
    in_maps = build_in_maps(x, Wg, W1, W2)
    if "nc" not in _NC_CACHE:
        _NC_CACHE["nc"] = build_kernel()
    from concourse.bass_utils import run_bass_kernel_spmd
    res = run_bass_kernel_spmd(_NC_CACHE["nc"], in_maps, list(range(R)))
    globals()['LAST_RES'] = res
    out = np.concatenate([np.asarray(res.results[r]["out_shard"]).astype(np.float32)
                          for r in range(R)], axis=0)
    return out.reshape(B, T, Dx)


if __name__ == "__main__":
    d = np.load("/tmp/inputs.npz")
    out = kernel(d["x"], d["Wg"], d["W1"], d["W2"])
    ref = np.load("/tmp/ref_out.npy")
    err = np.abs(out - ref).max() / np.abs(ref).max()
    print("rel err (absmax):", err)


# revision 32
# speedup vs baseline: 1.0642x; 1.0032x over previous
"""MoE MLP (top-2 of 8 experts) Trainium2 kernel — expert-parallel across 8 NeuronCores.

Strategy (v2):
  - Router data-parallel: each core computes logits for its 512-token shard with
    float32r PE matmuls (f32-exact numerics, 1 cycle/row), AllGathers a tiny
    per-token record [e1, e2, w1, w2] (4096 x 4 fp32).
  - Every core replicates the position computation (compact-slot assignment via
    triangular-matrix prefix-sum matmuls on the PE).
  - Each core owns ONE expert. It compacts (token_id, token_id, gating) triples
    via ONE indirect-DMA scatter (OOB-skip for unassigned), gathers the assigned
    token rows (bf16) from its HBM copy of x, transposes them to d-major via
    DMA-transpose on the SP queue, runs x@W1 -> relu^2 -> @W2 in bf16 on the PE,
    scales rows by the gating weight, and indirect-scatters the weighted rows
    into a host-zeroed dense (N_TOK+1, D) bf16 buffer at their token positions
    (padding slots land in the trash row N_TOK).
  - Combine: ONE ReduceScatter(add) over the dense buffers writes each core's
    512-token fp32-accumulated output shard directly (collective cost is
    out-bytes-based: 1 MB vs the 18.9 MB an AllGather of compacts would move).
  - PE kept warm with junk matmuls through the router/AllGather gaps so the
    p-state ramp doesn't tax the main matmuls.
"""
import sys, os
sys.path.insert(0, "/opt/trn_rl_repo")
import numpy as np
import ml_dtypes

import concourse.bass as bass
import concourse.bacc as bacc
import concourse.mybir as mybir
from concourse.tile import TileContext
from concourse.bass import IndirectOffsetOnAxis

P = 128
N_TOK = 4096      # B*T
D = 1024
E = 8
H = 2048
R = 8             # cores = experts
SH = N_TOK // R   # 512 tokens per shard
G = N_TOK // P    # 32 global 128-token chunks
GSH = G // R      # 4 chunks per shard
C = 1152          # expert capacity (max observed load 1091; binomial mean 1024, sd 28)
CB = C // P       # 9 capacity blocks
BIG = float(1 << 20)
F32 = mybir.dt.float32
F32R = mybir.dt.float32r
BF16 = mybir.dt.bfloat16
I32 = mybir.dt.int32

GROUPS = [(0, 512), (512, 512), (1024, 128)]   # mm slot groups (offset, width)
SCATTER_BATCH = 1      # g-chunks per indirect scatter instruction (1 = safe loop)
STRIDE0_ZERO = True
DMA_TRANSPOSE = True
XT_CHUNKS = 4          # router operand load split for early matmul start


def build_kernel():
    nc = bacc.Bacc(None, dynamic_dma_scratch_size=32768)

    # ---------------- I/O ----------------
    xT_shard = nc.declare_dram_parameter("xT_shard", [D, SH], F32, isOutput=False)
    x_bf = nc.declare_dram_parameter("x_bf", [N_TOK, D], BF16, isOutput=False)
    w1_in = nc.declare_dram_parameter("w1", [D, H], BF16, isOutput=False)
    w2_in = nc.declare_dram_parameter("w2", [H, D], BF16, isOutput=False)
    wg_in = nc.declare_dram_parameter("wg", [D, E], F32, isOutput=False)
    # constants
    ident_in = nc.declare_dram_parameter("ident", [P, P], F32, isOutput=False)
    lstrict_in = nc.declare_dram_parameter("lstrict", [P, P], F32, isOutput=False)  # [k,m]=1 iff k<m
    le00_in = nc.declare_dram_parameter("le00", [P, P], F32, isOutput=False)  # [(g',e'),(g,e)] e'==e & g'<g
    le01_in = nc.declare_dram_parameter("le01", [P, P], F32, isOutput=False)  # e'==e (all)
    iota8_in = nc.declare_dram_parameter("iota8", [P, E], F32, isOutput=False)   # rows = 0..7
    iotat_in = nc.declare_dram_parameter("iotat", [P, G], F32, isOutput=False)   # [p,g] = 128g+p
    onehr_in = nc.declare_dram_parameter("onehr", [P, E], F32, isOutput=False)   # rows = onehot(core)
    out_shard = nc.declare_dram_parameter("out_shard", [SH, D], BF16, isOutput=True)

    # ---------------- internal DRAM ----------------
    rec_own_d = nc.dram_tensor("rec_own_d", [SH, 4], F32)
    rec_all_d = nc.dram_tensor("rec_all_d", [N_TOK, 4], F32, addr_space="Shared")
    # compact buffer, device-prefilled with (gather_id=0, scatter_id=N_TOK, gating=0)
    comp = nc.dram_tensor("comp", [C, 3], F32)
    # dense per-core output buffers (column halves), device-zeroed; row N_TOK is
    # the padding trash row
    DH = D // 2
    y_half = [nc.dram_tensor("y_half%d" % h, [N_TOK + 1, DH], BF16) for h in range(2)]
    zrow_d = nc.dram_tensor("zrow_d", [1, D], BF16)
    y_rs = [nc.dram_tensor("y_rs%d" % h, [SH, DH], BF16) for h in range(2)]

    with TileContext(nc) as tc:
        with tc.tile_pool(name="const", bufs=1) as cp, \
             tc.tile_pool(name="wpool", bufs=1) as wp, \
             tc.tile_pool(name="sb", bufs=2) as sb, \
             tc.tile_pool(name="big", bufs=1) as bigp, \
             tc.tile_pool(name="ps", bufs=1, space="PSUM") as ps, \
             tc.tile_pool(name="mmps", bufs=3, space="PSUM") as mmps:

            # ---- early loads, spread over the three DMA-capable queues ----
            # SP: ident (warm fodder) then the router operand in chunks.
            wg_sb = cp.tile([P, D // P, E], F32)
            nc.sync.dma_start(out=wg_sb[:], in_=wg_in.rearrange('(dc p) e -> p dc e', p=P))
            ident = cp.tile([P, P], F32)
            nc.sync.dma_start(out=ident[:], in_=ident_in[:])
            DCC = (D // P) // XT_CHUNKS      # dc per chunk
            xT_c = [bigp.tile([P, DCC, SH], F32, name="xT_c%d" % k) for k in range(XT_CHUNKS)]
            xTr = xT_shard.rearrange('(dc p) t -> p dc t', p=P)
            for k in range(XT_CHUNKS):
                nc.sync.dma_start(out=xT_c[k][:], in_=xTr[:, k * DCC:(k + 1) * DCC, :])
            # Act: activation-table preload.
            ones_atl = cp.tile([1, 1], F32)
            nc.vector.memset(ones_atl[:], 1.0)
            atl = cp.tile([1, 1], F32)
            nc.scalar.activation(out=atl[:], in_=ones_atl[:], func=mybir.ActivationFunctionType.Sigmoid)
            iota8 = cp.tile([P, E], F32)
            nc.gpsimd.dma_start(out=iota8[:], in_=iota8_in[:])
            # Pool: position-machinery constants.
            lstrict = cp.tile([P, P], F32)
            nc.gpsimd.dma_start(out=lstrict[:], in_=lstrict_in[:])
            le00 = cp.tile([P, P], F32)
            nc.gpsimd.dma_start(out=le00[:], in_=le00_in[:])
            le01 = cp.tile([P, P], F32)
            nc.gpsimd.dma_start(out=le01[:], in_=le01_in[:])
            iotat = cp.tile([P, G], F32)
            nc.gpsimd.dma_start(out=iotat[:], in_=iotat_in[:])
            onehr = cp.tile([P, E], F32)
            nc.gpsimd.dma_start(out=onehr[:], in_=onehr_in[:])
            ones_1p = cp.tile([1, P], F32)
            nc.vector.memset(ones_1p[:], 1.0)
            ones_col = cp.tile([P, 1], F32)
            nc.vector.memset(ones_col[:], 1.0)
            identb = cp.tile([P, P], BF16)
            nc.vector.tensor_copy(out=identb[:], in_=ident[:])

            # ---- device-side init of comp prefill and the dense y buffer ----
            zb = cp.tile([P, D], BF16)
            nc.vector.memset(zb[:], 0.0)
            nc.scalar.dma_start(out=bass.AP(zrow_d, 0, [[D, 1], [1, D]]), in_=zb[0:1, :])
            for h in range(2):
                nc.scalar.dma_start(out=bass.AP(y_half[h], 0, [[DH, N_TOK + 1], [1, DH]]),
                                    in_=bass.AP(zrow_d, 0, [[0, N_TOK + 1], [1, DH]]))
            t3 = cp.tile([P, CB, 3], F32)
            nc.vector.memset(t3[:], 0.0)
            nc.vector.memset(t3[:, :, 1:2], float(N_TOK))
            nc.sync.dma_start(out=bass.AP(comp, 0, [[3, P], [3 * P, CB], [1, 3]]), in_=t3[:])

            # ---- PE warmup #1: keep the p-state ramp going until xT chunk 0 arrives ----
            warm_ps = ps.tile([P, 512], F32, space="PSUM", tag="warm")
            for _ in range(4):
                nc.tensor.matmul(out=warm_ps[:, 0:P], lhsT=ident[:], rhs=ident[:],
                                 start=True, stop=True, skip_group_check=True)

            # ---- router on own shard: f32 matmuls, chunk-pipelined with the load ----
            lgT_ps = ps.tile([E, SH], F32, space="PSUM", tag="pb")
            for k in range(XT_CHUNKS):
                for dck in range(DCC):
                    dc = k * DCC + dck
                    nc.tensor.matmul(out=lgT_ps[:], lhsT=wg_sb[:, dc, :], rhs=xT_c[k][:, dck, :],
                                     start=(dc == 0), stop=(dc == D // P - 1))
            lgT = sb.tile([E, SH], F32, tag="lgT")
            nc.vector.tensor_copy(out=lgT[:], in_=lgT_ps[:])
            logits = sb.tile([P, GSH, E], F32, tag="logits")
            for c in range(GSH):
                tp = ps.tile([P, E], F32, space="PSUM", tag="pb")
                nc.tensor.transpose(out=tp[:], in_=lgT[:, c * P:(c + 1) * P], identity=ident[:E, :E])
                nc.vector.tensor_copy(out=logits[:, c, :], in_=tp[:])

            mx = sb.tile([P, GSH, E], F32, tag="mx")
            for c in range(GSH):
                nc.vector.max(out=mx[:, c, :], in_=logits[:, c, :])
            m1 = mx[:, :, 0:1]
            m2 = mx[:, :, 1:2]
            dlt = sb.tile([P, GSH, 1], F32, tag="dlt")
            nc.vector.tensor_sub(out=dlt[:], in0=m1, in1=m2)
            rec_own = sb.tile([P, GSH, 4], F32, tag="rec_own")
            # w1 = sigmoid(m1-m2), w2 = sigmoid(m2-m1)
            nc.scalar.activation(out=rec_own[:, :, 2:3], in_=dlt[:], func=mybir.ActivationFunctionType.Sigmoid)
            nc.scalar.activation(out=rec_own[:, :, 3:4], in_=dlt[:], func=mybir.ActivationFunctionType.Sigmoid, scale=-1.0)
            # e1/e2 via onehot dot iota8
            oh = sb.tile([P, GSH, E], F32, tag="oh")
            tmp = sb.tile([P, GSH, E], F32, tag="ohtmp")
            nc.vector.tensor_tensor(out=oh[:], in0=logits[:], in1=m1.to_broadcast([P, GSH, E]),
                                    op=mybir.AluOpType.is_equal)
            nc.vector.tensor_tensor(out=tmp[:], in0=oh[:], in1=iota8[:].unsqueeze(1).to_broadcast([P, GSH, E]),
                                    op=mybir.AluOpType.mult)
            nc.vector.tensor_reduce(out=rec_own[:, :, 0:1], in_=tmp[:], axis=mybir.AxisListType.X,
                                    op=mybir.AluOpType.add)
            nc.vector.tensor_tensor(out=oh[:], in0=logits[:], in1=m2.to_broadcast([P, GSH, E]),
                                    op=mybir.AluOpType.is_equal)
            nc.vector.tensor_tensor(out=tmp[:], in0=oh[:], in1=iota8[:].unsqueeze(1).to_broadcast([P, GSH, E]),
                                    op=mybir.AluOpType.mult)
            nc.vector.tensor_reduce(out=rec_own[:, :, 1:2], in_=tmp[:], axis=mybir.AxisListType.X,
                                    op=mybir.AluOpType.add)
            # ship record on the Pool queue (SP is busy with w1): row t = 128c+p
            nc.gpsimd.dma_start(out=bass.AP(rec_own_d, 0, [[4, P], [SH, GSH], [1, 4]]), in_=rec_own[:])
            nc.gpsimd.collective_compute(
                "AllGather", mybir.AluOpType.bypass,
                ins=[rec_own_d[:]], outs=[rec_all_d[:]],
                replica_groups=[list(range(R))],
            )

            # w1 on SP right after the xT chunks; w2 on Act held past the record sigmoids.
            w1sb = wp.tile([P, D // P, H], BF16)   # [p, dc, h] = W1[dc*128+p, h]
            nc.sync.dma_start(out=w1sb[:], in_=w1_in.rearrange('(dc p) h -> p dc h', p=P))
            w2sb = wp.tile([P, H // P, D], BF16)   # [p, jj, d] = W2[jj*128+p, d]
            with tc.tile_wait_until(0.020):
                nc.scalar.dma_start(out=w2sb[:], in_=w2_in.rearrange('(jj p) d -> p jj d', p=P))

            # ---- PE warmup #2: cover the AllGather window ----
            with tc.tile_wait_until(0.0155):
                for i in range(24):
                    nc.tensor.matmul(out=warm_ps[0:E, :], lhsT=wg_sb[:, i % 8, :],
                                     rhs=xT_c[i % XT_CHUNKS][:, i % DCC, :],
                                     start=True, stop=True, skip_group_check=True)

            # ---- replicated positions over all tokens ----
            rec = sb.tile([P, G, 4], F32, tag="rec")
            nc.sync.dma_start(out=rec[:], in_=rec_all_d.rearrange('(g p) f -> p g f', p=P))
            e1a = rec[:, :, 0:1]
            e2a = rec[:, :, 1:2]
            w1a = rec[:, :, 2:3]
            w2a = rec[:, :, 3:4]
            oh1 = bigp.tile([P, G, E], F32)
            oh2 = bigp.tile([P, G, E], F32)
            i8b = iota8[:].unsqueeze(1).to_broadcast([P, G, E])
            nc.vector.tensor_tensor(out=oh1[:], in0=e1a.to_broadcast([P, G, E]), in1=i8b, op=mybir.AluOpType.is_equal)
            nc.vector.tensor_tensor(out=oh2[:], in0=e2a.to_broadcast([P, G, E]), in1=i8b, op=mybir.AluOpType.is_equal)
            mask = bigp.tile([P, G, E], F32)
            nc.vector.tensor_add(out=mask[:], in0=oh1[:], in1=oh2[:])
            mask2 = mask[:].rearrange('p g e -> p (g e)')

            pos_ps = ps.tile([P, G * E], F32, space="PSUM", tag="pe")
            nc.tensor.matmul(out=pos_ps[:], lhsT=lstrict[:], rhs=mask2, start=True, stop=False)
            # totals per (g,e), partition-major halves
            t0_ps = ps.tile([P, 1], F32, space="PSUM", tag="pb")
            nc.tensor.matmul(out=t0_ps[:], lhsT=mask2[:, 0:P], rhs=ones_col[:], start=True, stop=True)
            t1_ps = ps.tile([P, 1], F32, space="PSUM", tag="pb")
            nc.tensor.matmul(out=t1_ps[:], lhsT=mask2[:, P:2 * P], rhs=ones_col[:], start=True, stop=True)
            t0 = sb.tile([P, 1], F32, tag="t0sb")
            nc.vector.tensor_copy(out=t0[:], in_=t0_ps[:])
            t1 = sb.tile([P, 1], F32, tag="t1sb")
            nc.vector.tensor_copy(out=t1[:], in_=t1_ps[:])
            off0_ps = ps.tile([P, 1], F32, space="PSUM", tag="pb")
            nc.tensor.matmul(out=off0_ps[:], lhsT=le00[:], rhs=t0[:], start=True, stop=True)
            off1_ps = ps.tile([P, 1], F32, space="PSUM", tag="pb")
            nc.tensor.matmul(out=off1_ps[:], lhsT=le01[:], rhs=t0[:], start=True, stop=False)
            nc.tensor.matmul(out=off1_ps[:], lhsT=le00[:], rhs=t1[:], start=False, stop=True)
            off0 = sb.tile([P, 1], F32, tag="off0sb")
            nc.vector.tensor_copy(out=off0[:], in_=off0_ps[:])
            off1 = sb.tile([P, 1], F32, tag="off1sb")
            nc.vector.tensor_copy(out=off1[:], in_=off1_ps[:])
            offT_ps = ps.tile([1, P], F32, space="PSUM", tag="pb")
            offs_1p = sb.tile([1, 2 * P], F32, tag="offs1p")
            nc.tensor.transpose(out=offT_ps[:], in_=off0[:], identity=ident[:])
            nc.vector.tensor_copy(out=offs_1p[:, 0:P], in_=offT_ps[:])
            offT2_ps = ps.tile([1, P], F32, space="PSUM", tag="pb")
            nc.tensor.transpose(out=offT2_ps[:], in_=off1[:], identity=ident[:])
            nc.vector.tensor_copy(out=offs_1p[:, P:2 * P], in_=offT2_ps[:])
            # replicate chunk offsets to all partitions, accumulating into pos_ps
            nc.tensor.matmul(out=pos_ps[:], lhsT=ones_1p[:], rhs=offs_1p[:], start=False, stop=True)
            pos = bigp.tile([P, G, E], F32)
            nc.vector.tensor_copy(out=pos[:], in_=pos_ps[:].rearrange('p (g e) -> p g e', g=G))

            # ---- producer: gating + one-shot scatter compaction for own expert ----
            st = bigp.tile([P, G, E], F32)
            isr1 = sb.tile([P, G], F32, tag="isr1")
            isr2 = sb.tile([P, G], F32, tag="isr2")
            ohrb = onehr[:].unsqueeze(1).to_broadcast([P, G, E])
            nc.vector.tensor_tensor(out=st[:], in0=oh1[:], in1=ohrb, op=mybir.AluOpType.mult)
            nc.vector.tensor_reduce(out=isr1[:], in_=st[:], axis=mybir.AxisListType.X, op=mybir.AluOpType.add)
            nc.vector.tensor_tensor(out=st[:], in0=oh2[:], in1=ohrb, op=mybir.AluOpType.mult)
            nc.vector.tensor_reduce(out=isr2[:], in_=st[:], axis=mybir.AxisListType.X, op=mybir.AluOpType.add)
            g_r = sb.tile([P, G], F32, tag="g_r")
            tmpg2 = sb.tile([P, G], F32, tag="tmpg2")
            nc.vector.tensor_tensor(out=g_r[:], in0=isr1[:], in1=w1a.rearrange('p g o -> p (g o)'), op=mybir.AluOpType.mult)
            nc.vector.tensor_tensor(out=tmpg2[:], in0=isr2[:], in1=w2a.rearrange('p g o -> p (g o)'), op=mybir.AluOpType.mult)
            nc.vector.tensor_add(out=g_r[:], in0=g_r[:], in1=tmpg2[:])
            maskr = sb.tile([P, G], F32, tag="maskr")
            nc.vector.tensor_add(out=maskr[:], in0=isr1[:], in1=isr2[:])
            pos_r = sb.tile([P, G], F32, tag="pos_r")
            nc.vector.tensor_tensor(out=st[:], in0=mask[:], in1=ohrb, op=mybir.AluOpType.mult)
            nc.vector.tensor_tensor(out=st[:], in0=st[:], in1=pos[:], op=mybir.AluOpType.mult)
            nc.vector.tensor_reduce(out=pos_r[:], in_=st[:], axis=mybir.AxisListType.X, op=mybir.AluOpType.add)
            # scatter offsets: pos_r + BIG*(1-maskr)
            offsc = sb.tile([P, G], F32, tag="offsc")
            nc.vector.tensor_scalar_mul(tmpg2[:], maskr[:], -BIG)
            nc.vector.tensor_scalar_add(offsc[:], tmpg2[:], BIG)
            nc.vector.tensor_add(out=offsc[:], in0=offsc[:], in1=pos_r[:])
            offsc_i = sb.tile([P, G], I32, tag="offsci")
            nc.vector.tensor_copy(out=offsc_i[:], in_=offsc[:])
            vals = sb.tile([P, G, 3], F32, tag="vals")
            nc.vector.tensor_copy(out=vals[:, :, 0], in_=iotat[:])
            nc.vector.tensor_copy(out=vals[:, :, 1], in_=iotat[:])
            nc.vector.tensor_copy(out=vals[:, :, 2], in_=g_r[:])
            vals2 = vals[:].rearrange('p g f -> p (g f)')
            # scatter the (token, token, gating) triples, OOB-skip unassigned
            for g in range(0, G, SCATTER_BATCH):
                src = vals[:, g, :] if SCATTER_BATCH == 1 else vals2[:, g * 3:(g + SCATTER_BATCH) * 3]
                nc.gpsimd.indirect_dma_start(
                    out=comp[:],
                    out_offset=IndirectOffsetOnAxis(ap=offsc_i[:, g:g + SCATTER_BATCH], axis=0),
                    in_=src, in_offset=None,
                    bounds_check=C - 1, oob_is_err=False,
                )
            # reload compact ids & gatings (padding rows keep host prefill: 0 / N_TOK / 0)
            ids_f = sb.tile([P, CB], F32, tag="idsf")
            nc.scalar.dma_start(out=ids_f[:], in_=bass.AP(comp, 0, [[3, P], [3 * P, CB]]))
            idsc_f = sb.tile([P, CB], F32, tag="idscf")
            nc.sync.dma_start(out=idsc_f[:], in_=bass.AP(comp, 1, [[3, P], [3 * P, CB]]))
            g_load = sb.tile([P, CB], F32, tag="gload")
            nc.scalar.dma_start(out=g_load[:], in_=bass.AP(comp, 2, [[3, P], [3 * P, CB]]))
            ids_i = sb.tile([P, CB], I32, tag="idsi")
            nc.vector.tensor_copy(out=ids_i[:], in_=ids_f[:])
            idsc_i = sb.tile([P, CB], I32, tag="idsci")
            nc.vector.tensor_copy(out=idsc_i[:], in_=idsc_f[:])

            # ---- gather x rows (token-major), DMA-transpose to d-major per group ----
            xTg = [bigp.tile([P, D // P, n], BF16, name="xTg_%d" % gi)
                   for gi, (o, n) in enumerate(GROUPS)]
            for c in range(CB):
                gi = 0 if c < 4 else (1 if c < 8 else 2)
                o = GROUPS[gi][0]
                lc = c * P - o   # column offset within group tile
                xg_c = bigp.tile([P, D], BF16, tag="xgc", name="xg_%d" % c, bufs=6)
                nc.gpsimd.indirect_dma_start(
                    out=xg_c[:], out_offset=None,
                    in_=x_bf[:],
                    in_offset=IndirectOffsetOnAxis(ap=ids_i[:, c:c + 1], axis=0),
                )
                for dc in range(D // P):
                    tps = ps.tile([P, P], BF16, space="PSUM", tag="rot", bufs=2,
                                  name="tps_%d_%d" % (c, dc))
                    nc.tensor.transpose(out=tps[:], in_=xg_c[:, dc * P:(dc + 1) * P], identity=identb[:])
                    nc.vector.tensor_copy(out=xTg[gi][:, dc, lc:lc + P], in_=tps[:])


            # ---- PE warmup #3: bridge the scatter/gather window up to mm1 ----
            with tc.tile_wait_until(0.040):
                for i in range(24):
                    nc.tensor.matmul(out=warm_ps[0:E, :], lhsT=wg_sb[:, i % 8, :],
                                     rhs=xT_c[i % XT_CHUNKS][:, i % DCC, :],
                                     start=True, stop=True, skip_group_check=True)

            # ---- mm1: hT[j] = relu(x W1)^2, h-major, per slot group ----
            hT = [bigp.tile([P, H // P, n], BF16, name="hT_%d" % gi)
                  for gi, (o, n) in enumerate(GROUPS)]
            for gi, (o, n) in enumerate(GROUPS):
                for j in range(H // P):
                    hps = mmps.tile([P, 512], F32, space="PSUM", tag="mm",
                                    name="hps_%d_%d" % (gi, j))
                    for dc in range(D // P):
                        nc.tensor.matmul(out=hps[:, :n], lhsT=w1sb[:, dc, j * P:(j + 1) * P],
                                         rhs=xTg[gi][:, dc, :],
                                         start=(dc == 0), stop=(dc == D // P - 1))
                    rl = sb.tile([P, 512], F32, tag="rl", name="rl_%d_%d" % (gi, j), bufs=4)
                    nc.scalar.activation(out=rl[:, :n], in_=hps[:, :n], func=mybir.ActivationFunctionType.Relu)
                    nc.vector.tensor_tensor(out=hT[gi][:, j, :], in0=rl[:, :n], in1=rl[:, :n],
                                            op=mybir.AluOpType.mult)

            # ---- mm2: y = hT^T W2, half-column passes so RS(half0) overlaps pass 1 ----
            for dn in range(2):
                for m in range(CB):
                    gi = 0 if m < 4 else (1 if m < 8 else 2)
                    o = GROUPS[gi][0]
                    lm = m * P - o
                    yrow = sb.tile([P, DH], BF16, tag="yrow", name="yrow_%d_%d" % (dn, m), bufs=12)
                    yps = mmps.tile([P, 512], F32, space="PSUM", tag="mm",
                                    name="yps_%d_%d" % (m, dn))
                    for jj in range(H // P):
                        nc.tensor.matmul(out=yps[:], lhsT=hT[gi][:, jj, lm:lm + P],
                                         rhs=w2sb[:, jj, dn * DH:(dn + 1) * DH],
                                         start=(jj == 0), stop=(jj == H // P - 1))
                    nc.scalar.activation(out=yrow[:], in_=yps[:],
                                         func=mybir.ActivationFunctionType.Copy,
                                         scale=g_load[:, m:m + 1])
                    nc.gpsimd.indirect_dma_start(
                        out=y_half[dn][:],
                        out_offset=IndirectOffsetOnAxis(ap=idsc_i[:, m:m + 1], axis=0),
                        in_=yrow[:], in_offset=None,
                    )
                # combine this half: ReduceScatter(add) overlaps the next pass
                nc.gpsimd.collective_compute(
                    "ReduceScatter", mybir.AluOpType.add,
                    ins=[y_half[dn][0:N_TOK, :]], outs=[y_rs[dn][:]],
                    replica_groups=[list(range(R))],
                )
                eng = nc.sync if dn == 0 else nc.scalar
                eng.dma_start(out=bass.AP(out_shard, dn * DH, [[D, SH], [1, DH]]),
                              in_=bass.AP(y_rs[dn], 0, [[DH, SH], [1, DH]]))

    nc.finalize()
    return nc


# ---------------- host-side constants ----------------
def host_constants():
    ident = np.eye(P, dtype=np.float32)
    lstrict = np.triu(np.ones((P, P), np.float32), k=1)  # [k, m] = 1 iff m > k
    # rows/cols indexed by (g*8 + e) within a 128-slot half (16 g values)
    gg, ee = np.arange(16), np.arange(E)
    gi = np.repeat(gg, E)   # g of row index
    ei = np.tile(ee, 16)    # e of row index
    le00 = ((ei[:, None] == ei[None, :]) & (gi[:, None] < gi[None, :])).astype(np.float32)
    le01 = (ei[:, None] == ei[None, :]).astype(np.float32)
    iota8 = np.broadcast_to(np.arange(E, dtype=np.float32), (P, E)).copy()
    iotat = (np.arange(G, dtype=np.float32)[None, :] * P + np.arange(P, dtype=np.float32)[:, None]).copy()
    return ident, lstrict, le00, le01, iota8, iotat


def build_in_maps(x, Wg, W1, W2):
    x = np.asarray(x); Wg = np.asarray(Wg); W1 = np.asarray(W1); W2 = np.asarray(W2)
    xt = x.reshape(N_TOK, D).astype(np.float32)
    x_bf = xt.astype(ml_dtypes.bfloat16)
    ident, lstrict, le00, le01, iota8, iotat = host_constants()
    in_maps = []
    for r in range(R):
        onehr = np.zeros((P, E), np.float32); onehr[:, r] = 1.0
        in_maps.append({
            "xT_shard": np.ascontiguousarray(xt[r * SH:(r + 1) * SH, :].T),
            "x_bf": x_bf,
            "w1": W1[r].astype(ml_dtypes.bfloat16),
            "w2": W2[r].astype(ml_dtypes.bfloat16),
            "wg": Wg.astype(np.float32),
            "ident": ident, "lstrict": lstrict, "le00": le00, "le01": le01,
            "iota8": iota8, "iotat": iotat, "onehr": onehr,
        })
    return in_maps


_NC_CACHE = {}

def kernel(x, Wg, W1, W2):
    x = np.asarray(x)
    B, T, Dx = x.shape
    in_maps = build_in_maps(x, Wg, W1, W2)
    if "nc" not in _NC_CACHE:
        _NC_CACHE["nc"] = build_kernel()
    from concourse.bass_utils import run_bass_kernel_spmd
    res = run_bass_kernel_spmd(_NC_CACHE["nc"], in_maps, list(range(R)))
    globals()['LAST_RES'] = res
    out = np.concatenate([np.asarray(res.results[r]["out_shard"]).astype(np.float32)
                          for r in range(R)], axis=0)
    return out.reshape(B, T, Dx)


if __name__ == "__main__":
    d = np.load("/tmp/inputs.npz")
    out = kernel(d["x"], d["Wg"], d["W1"], d["W2"])
    ref = np.load("/tmp/ref_out.npy")
    err = np.abs(out - ref).max() / np.abs(ref).max()
    print("rel err (absmax):", err)


# revision 33
# speedup vs baseline: 1.0741x; 1.0093x over previous
"""MoE MLP (top-2 of 8 experts) Trainium2 kernel — expert-parallel across 8 NeuronCores.

Strategy (v2):
  - Router data-parallel: each core computes logits for its 512-token shard with
    float32r PE matmuls (f32-exact numerics, 1 cycle/row), AllGathers a tiny
    per-token record [e1, e2, w1, w2] (4096 x 4 fp32).
  - Every core replicates the position computation (compact-slot assignment via
    triangular-matrix prefix-sum matmuls on the PE).
  - Each core owns ONE expert. It compacts (token_id, token_id, gating) triples
    via ONE indirect-DMA scatter (OOB-skip for unassigned), gathers the assigned
    token rows (bf16) from its HBM copy of x, transposes them to d-major via
    DMA-transpose on the SP queue, runs x@W1 -> relu^2 -> @W2 in bf16 on the PE,
    scales rows by the gating weight, and indirect-scatters the weighted rows
    into a host-zeroed dense (N_TOK+1, D) bf16 buffer at their token positions
    (padding slots land in the trash row N_TOK).
  - Combine: ONE ReduceScatter(add) over the dense buffers writes each core's
    512-token fp32-accumulated output shard directly (collective cost is
    out-bytes-based: 1 MB vs the 18.9 MB an AllGather of compacts would move).
  - PE kept warm with junk matmuls through the router/AllGather gaps so the
    p-state ramp doesn't tax the main matmuls.
"""
import sys, os
sys.path.insert(0, "/opt/trn_rl_repo")
import numpy as np
import ml_dtypes

import concourse.bass as bass
import concourse.bacc as bacc
import concourse.mybir as mybir
from concourse.tile import TileContext
from concourse.bass import IndirectOffsetOnAxis

P = 128
N_TOK = 4096      # B*T
D = 1024
E = 8
H = 2048
R = 8             # cores = experts
SH = N_TOK // R   # 512 tokens per shard
G = N_TOK // P    # 32 global 128-token chunks
GSH = G // R      # 4 chunks per shard
C = 1152          # expert capacity (max observed load 1091; binomial mean 1024, sd 28)
CB = C // P       # 9 capacity blocks
BIG = float(1 << 20)
F32 = mybir.dt.float32
F32R = mybir.dt.float32r
BF16 = mybir.dt.bfloat16
I32 = mybir.dt.int32

GROUPS = [(0, 256), (256, 384), (640, 512)]   # mm slot groups (offset, width)
SCATTER_BATCH = 1      # g-chunks per indirect scatter instruction (1 = safe loop)
STRIDE0_ZERO = True
DMA_TRANSPOSE = True
XT_CHUNKS = 4          # router operand load split for early matmul start


def build_kernel():
    nc = bacc.Bacc(None, dynamic_dma_scratch_size=32768)

    # ---------------- I/O ----------------
    xT_shard = nc.declare_dram_parameter("xT_shard", [D, SH], F32, isOutput=False)
    x_bf = nc.declare_dram_parameter("x_bf", [N_TOK, D], BF16, isOutput=False)
    w1_in = nc.declare_dram_parameter("w1", [D, H], BF16, isOutput=False)
    w2_in = nc.declare_dram_parameter("w2", [H, D], BF16, isOutput=False)
    wg_in = nc.declare_dram_parameter("wg", [D, E], F32, isOutput=False)
    # constants
    ident_in = nc.declare_dram_parameter("ident", [P, P], F32, isOutput=False)
    lstrict_in = nc.declare_dram_parameter("lstrict", [P, P], F32, isOutput=False)  # [k,m]=1 iff k<m
    le00_in = nc.declare_dram_parameter("le00", [P, P], F32, isOutput=False)  # [(g',e'),(g,e)] e'==e & g'<g
    le01_in = nc.declare_dram_parameter("le01", [P, P], F32, isOutput=False)  # e'==e (all)
    iota8_in = nc.declare_dram_parameter("iota8", [P, E], F32, isOutput=False)   # rows = 0..7
    iotat_in = nc.declare_dram_parameter("iotat", [P, G], F32, isOutput=False)   # [p,g] = 128g+p
    onehr_in = nc.declare_dram_parameter("onehr", [P, E], F32, isOutput=False)   # rows = onehot(core)
    out_shard = nc.declare_dram_parameter("out_shard", [SH, D], BF16, isOutput=True)

    # ---------------- internal DRAM ----------------
    rec_own_d = nc.dram_tensor("rec_own_d", [SH, 4], F32)
    rec_all_d = nc.dram_tensor("rec_all_d", [N_TOK, 4], F32, addr_space="Shared")
    # compact buffer, device-prefilled with (gather_id=0, scatter_id=N_TOK, gating=0)
    comp = nc.dram_tensor("comp", [C, 3], F32)
    # dense per-core output buffers (column halves), device-zeroed; row N_TOK is
    # the padding trash row
    DH = D // 2
    y_half = [nc.dram_tensor("y_half%d" % h, [N_TOK + 1, DH], BF16) for h in range(2)]
    zrow_d = nc.dram_tensor("zrow_d", [1, D], BF16)
    y_rs = [nc.dram_tensor("y_rs%d" % h, [SH, DH], BF16) for h in range(2)]

    with TileContext(nc) as tc:
        with tc.tile_pool(name="const", bufs=1) as cp, \
             tc.tile_pool(name="wpool", bufs=1) as wp, \
             tc.tile_pool(name="sb", bufs=2) as sb, \
             tc.tile_pool(name="big", bufs=1) as bigp, \
             tc.tile_pool(name="ps", bufs=1, space="PSUM") as ps, \
             tc.tile_pool(name="mmps", bufs=3, space="PSUM") as mmps:

            # ---- early loads, spread over the three DMA-capable queues ----
            # SP: ident (warm fodder) then the router operand in chunks.
            wg_sb = cp.tile([P, D // P, E], F32)
            nc.sync.dma_start(out=wg_sb[:], in_=wg_in.rearrange('(dc p) e -> p dc e', p=P))
            ident = cp.tile([P, P], F32)
            nc.sync.dma_start(out=ident[:], in_=ident_in[:])
            DCC = (D // P) // XT_CHUNKS      # dc per chunk
            xT_c = [bigp.tile([P, DCC, SH], F32, name="xT_c%d" % k) for k in range(XT_CHUNKS)]
            xTr = xT_shard.rearrange('(dc p) t -> p dc t', p=P)
            for k in range(XT_CHUNKS):
                nc.sync.dma_start(out=xT_c[k][:], in_=xTr[:, k * DCC:(k + 1) * DCC, :])
            # Act: activation-table preload.
            ones_atl = cp.tile([1, 1], F32)
            nc.vector.memset(ones_atl[:], 1.0)
            atl = cp.tile([1, 1], F32)
            nc.scalar.activation(out=atl[:], in_=ones_atl[:], func=mybir.ActivationFunctionType.Sigmoid)
            iota8 = cp.tile([P, E], F32)
            nc.gpsimd.dma_start(out=iota8[:], in_=iota8_in[:])
            # Pool: position-machinery constants.
            lstrict = cp.tile([P, P], F32)
            nc.gpsimd.dma_start(out=lstrict[:], in_=lstrict_in[:])
            le00 = cp.tile([P, P], F32)
            nc.gpsimd.dma_start(out=le00[:], in_=le00_in[:])
            le01 = cp.tile([P, P], F32)
            nc.gpsimd.dma_start(out=le01[:], in_=le01_in[:])
            iotat = cp.tile([P, G], F32)
            nc.gpsimd.dma_start(out=iotat[:], in_=iotat_in[:])
            onehr = cp.tile([P, E], F32)
            nc.gpsimd.dma_start(out=onehr[:], in_=onehr_in[:])
            ones_1p = cp.tile([1, P], F32)
            nc.vector.memset(ones_1p[:], 1.0)
            ones_col = cp.tile([P, 1], F32)
            nc.vector.memset(ones_col[:], 1.0)
            identb = cp.tile([P, P], BF16)
            nc.vector.tensor_copy(out=identb[:], in_=ident[:])

            # ---- device-side init of comp prefill and the dense y buffer ----
            zb = cp.tile([P, D], BF16)
            nc.vector.memset(zb[:], 0.0)
            nc.scalar.dma_start(out=bass.AP(zrow_d, 0, [[D, 1], [1, D]]), in_=zb[0:1, :])
            for h in range(2):
                nc.scalar.dma_start(out=bass.AP(y_half[h], 0, [[DH, N_TOK + 1], [1, DH]]),
                                    in_=bass.AP(zrow_d, 0, [[0, N_TOK + 1], [1, DH]]))
            t3 = cp.tile([P, CB, 3], F32)
            nc.vector.memset(t3[:], 0.0)
            nc.vector.memset(t3[:, :, 1:2], float(N_TOK))
            nc.sync.dma_start(out=bass.AP(comp, 0, [[3, P], [3 * P, CB], [1, 3]]), in_=t3[:])

            # ---- PE warmup #1: keep the p-state ramp going until xT chunk 0 arrives ----
            warm_ps = ps.tile([P, 512], F32, space="PSUM", tag="warm")
            for _ in range(4):
                nc.tensor.matmul(out=warm_ps[:, 0:P], lhsT=ident[:], rhs=ident[:],
                                 start=True, stop=True, skip_group_check=True)

            # ---- router on own shard: f32 matmuls, chunk-pipelined with the load ----
            lgT_ps = ps.tile([E, SH], F32, space="PSUM", tag="pb")
            for k in range(XT_CHUNKS):
                for dck in range(DCC):
                    dc = k * DCC + dck
                    nc.tensor.matmul(out=lgT_ps[:], lhsT=wg_sb[:, dc, :], rhs=xT_c[k][:, dck, :],
                                     start=(dc == 0), stop=(dc == D // P - 1))
            lgT = sb.tile([E, SH], F32, tag="lgT")
            nc.vector.tensor_copy(out=lgT[:], in_=lgT_ps[:])
            logits = sb.tile([P, GSH, E], F32, tag="logits")
            for c in range(GSH):
                tp = ps.tile([P, E], F32, space="PSUM", tag="pb")
                nc.tensor.transpose(out=tp[:], in_=lgT[:, c * P:(c + 1) * P], identity=ident[:E, :E])
                nc.vector.tensor_copy(out=logits[:, c, :], in_=tp[:])

            mx = sb.tile([P, GSH, E], F32, tag="mx")
            for c in range(GSH):
                nc.vector.max(out=mx[:, c, :], in_=logits[:, c, :])
            m1 = mx[:, :, 0:1]
            m2 = mx[:, :, 1:2]
            dlt = sb.tile([P, GSH, 1], F32, tag="dlt")
            nc.vector.tensor_sub(out=dlt[:], in0=m1, in1=m2)
            rec_own = sb.tile([P, GSH, 4], F32, tag="rec_own")
            # w1 = sigmoid(m1-m2), w2 = sigmoid(m2-m1)
            nc.scalar.activation(out=rec_own[:, :, 2:3], in_=dlt[:], func=mybir.ActivationFunctionType.Sigmoid)
            nc.scalar.activation(out=rec_own[:, :, 3:4], in_=dlt[:], func=mybir.ActivationFunctionType.Sigmoid, scale=-1.0)
            # e1/e2 via onehot dot iota8
            oh = sb.tile([P, GSH, E], F32, tag="oh")
            tmp = sb.tile([P, GSH, E], F32, tag="ohtmp")
            nc.vector.tensor_tensor(out=oh[:], in0=logits[:], in1=m1.to_broadcast([P, GSH, E]),
                                    op=mybir.AluOpType.is_equal)
            nc.vector.tensor_tensor(out=tmp[:], in0=oh[:], in1=iota8[:].unsqueeze(1).to_broadcast([P, GSH, E]),
                                    op=mybir.AluOpType.mult)
            nc.vector.tensor_reduce(out=rec_own[:, :, 0:1], in_=tmp[:], axis=mybir.AxisListType.X,
                                    op=mybir.AluOpType.add)
            nc.vector.tensor_tensor(out=oh[:], in0=logits[:], in1=m2.to_broadcast([P, GSH, E]),
                                    op=mybir.AluOpType.is_equal)
            nc.vector.tensor_tensor(out=tmp[:], in0=oh[:], in1=iota8[:].unsqueeze(1).to_broadcast([P, GSH, E]),
                                    op=mybir.AluOpType.mult)
            nc.vector.tensor_reduce(out=rec_own[:, :, 1:2], in_=tmp[:], axis=mybir.AxisListType.X,
                                    op=mybir.AluOpType.add)
            # ship record on the Pool queue (SP is busy with w1): row t = 128c+p
            nc.gpsimd.dma_start(out=bass.AP(rec_own_d, 0, [[4, P], [SH, GSH], [1, 4]]), in_=rec_own[:])
            nc.gpsimd.collective_compute(
                "AllGather", mybir.AluOpType.bypass,
                ins=[rec_own_d[:]], outs=[rec_all_d[:]],
                replica_groups=[list(range(R))],
            )

            # w1 on SP right after the xT chunks; w2 on Act held past the record sigmoids.
            w1sb = wp.tile([P, D // P, H], BF16)   # [p, dc, h] = W1[dc*128+p, h]
            nc.sync.dma_start(out=w1sb[:], in_=w1_in.rearrange('(dc p) h -> p dc h', p=P))
            w2sb = wp.tile([P, H // P, D], BF16)   # [p, jj, d] = W2[jj*128+p, d]
            with tc.tile_wait_until(0.020):
                nc.scalar.dma_start(out=w2sb[:], in_=w2_in.rearrange('(jj p) d -> p jj d', p=P))

            # ---- PE warmup #2: cover the AllGather window ----
            with tc.tile_wait_until(0.0155):
                for i in range(24):
                    nc.tensor.matmul(out=warm_ps[0:E, :], lhsT=wg_sb[:, i % 8, :],
                                     rhs=xT_c[i % XT_CHUNKS][:, i % DCC, :],
                                     start=True, stop=True, skip_group_check=True)

            # ---- replicated positions over all tokens ----
            rec = sb.tile([P, G, 4], F32, tag="rec")
            nc.sync.dma_start(out=rec[:], in_=rec_all_d.rearrange('(g p) f -> p g f', p=P))
            e1a = rec[:, :, 0:1]
            e2a = rec[:, :, 1:2]
            w1a = rec[:, :, 2:3]
            w2a = rec[:, :, 3:4]
            oh1 = bigp.tile([P, G, E], F32)
            oh2 = bigp.tile([P, G, E], F32)
            i8b = iota8[:].unsqueeze(1).to_broadcast([P, G, E])
            nc.vector.tensor_tensor(out=oh1[:], in0=e1a.to_broadcast([P, G, E]), in1=i8b, op=mybir.AluOpType.is_equal)
            nc.vector.tensor_tensor(out=oh2[:], in0=e2a.to_broadcast([P, G, E]), in1=i8b, op=mybir.AluOpType.is_equal)
            mask = bigp.tile([P, G, E], F32)
            nc.vector.tensor_add(out=mask[:], in0=oh1[:], in1=oh2[:])
            mask2 = mask[:].rearrange('p g e -> p (g e)')

            pos_ps = ps.tile([P, G * E], F32, space="PSUM", tag="pe")
            nc.tensor.matmul(out=pos_ps[:], lhsT=lstrict[:], rhs=mask2, start=True, stop=False)
            # totals per (g,e), partition-major halves
            t0_ps = ps.tile([P, 1], F32, space="PSUM", tag="pb")
            nc.tensor.matmul(out=t0_ps[:], lhsT=mask2[:, 0:P], rhs=ones_col[:], start=True, stop=True)
            t1_ps = ps.tile([P, 1], F32, space="PSUM", tag="pb")
            nc.tensor.matmul(out=t1_ps[:], lhsT=mask2[:, P:2 * P], rhs=ones_col[:], start=True, stop=True)
            t0 = sb.tile([P, 1], F32, tag="t0sb")
            nc.vector.tensor_copy(out=t0[:], in_=t0_ps[:])
            t1 = sb.tile([P, 1], F32, tag="t1sb")
            nc.vector.tensor_copy(out=t1[:], in_=t1_ps[:])
            off0_ps = ps.tile([P, 1], F32, space="PSUM", tag="pb")
            nc.tensor.matmul(out=off0_ps[:], lhsT=le00[:], rhs=t0[:], start=True, stop=True)
            off1_ps = ps.tile([P, 1], F32, space="PSUM", tag="pb")
            nc.tensor.matmul(out=off1_ps[:], lhsT=le01[:], rhs=t0[:], start=True, stop=False)
            nc.tensor.matmul(out=off1_ps[:], lhsT=le00[:], rhs=t1[:], start=False, stop=True)
            off0 = sb.tile([P, 1], F32, tag="off0sb")
            nc.vector.tensor_copy(out=off0[:], in_=off0_ps[:])
            off1 = sb.tile([P, 1], F32, tag="off1sb")
            nc.vector.tensor_copy(out=off1[:], in_=off1_ps[:])
            offT_ps = ps.tile([1, P], F32, space="PSUM", tag="pb")
            offs_1p = sb.tile([1, 2 * P], F32, tag="offs1p")
            nc.tensor.transpose(out=offT_ps[:], in_=off0[:], identity=ident[:])
            nc.vector.tensor_copy(out=offs_1p[:, 0:P], in_=offT_ps[:])
            offT2_ps = ps.tile([1, P], F32, space="PSUM", tag="pb")
            nc.tensor.transpose(out=offT2_ps[:], in_=off1[:], identity=ident[:])
            nc.vector.tensor_copy(out=offs_1p[:, P:2 * P], in_=offT2_ps[:])
            # replicate chunk offsets to all partitions, accumulating into pos_ps
            nc.tensor.matmul(out=pos_ps[:], lhsT=ones_1p[:], rhs=offs_1p[:], start=False, stop=True)
            pos = bigp.tile([P, G, E], F32)
            nc.vector.tensor_copy(out=pos[:], in_=pos_ps[:].rearrange('p (g e) -> p g e', g=G))

            # ---- producer: gating + one-shot scatter compaction for own expert ----
            st = bigp.tile([P, G, E], F32)
            isr1 = sb.tile([P, G], F32, tag="isr1")
            isr2 = sb.tile([P, G], F32, tag="isr2")
            ohrb = onehr[:].unsqueeze(1).to_broadcast([P, G, E])
            nc.vector.tensor_tensor(out=st[:], in0=oh1[:], in1=ohrb, op=mybir.AluOpType.mult)
            nc.vector.tensor_reduce(out=isr1[:], in_=st[:], axis=mybir.AxisListType.X, op=mybir.AluOpType.add)
            nc.vector.tensor_tensor(out=st[:], in0=oh2[:], in1=ohrb, op=mybir.AluOpType.mult)
            nc.vector.tensor_reduce(out=isr2[:], in_=st[:], axis=mybir.AxisListType.X, op=mybir.AluOpType.add)
            g_r = sb.tile([P, G], F32, tag="g_r")
            tmpg2 = sb.tile([P, G], F32, tag="tmpg2")
            nc.vector.tensor_tensor(out=g_r[:], in0=isr1[:], in1=w1a.rearrange('p g o -> p (g o)'), op=mybir.AluOpType.mult)
            nc.vector.tensor_tensor(out=tmpg2[:], in0=isr2[:], in1=w2a.rearrange('p g o -> p (g o)'), op=mybir.AluOpType.mult)
            nc.vector.tensor_add(out=g_r[:], in0=g_r[:], in1=tmpg2[:])
            maskr = sb.tile([P, G], F32, tag="maskr")
            nc.vector.tensor_add(out=maskr[:], in0=isr1[:], in1=isr2[:])
            pos_r = sb.tile([P, G], F32, tag="pos_r")
            nc.vector.tensor_tensor(out=st[:], in0=mask[:], in1=ohrb, op=mybir.AluOpType.mult)
            nc.vector.tensor_tensor(out=st[:], in0=st[:], in1=pos[:], op=mybir.AluOpType.mult)
            nc.vector.tensor_reduce(out=pos_r[:], in_=st[:], axis=mybir.AxisListType.X, op=mybir.AluOpType.add)
            # scatter offsets: pos_r + BIG*(1-maskr)
            offsc = sb.tile([P, G], F32, tag="offsc")
            nc.vector.tensor_scalar_mul(tmpg2[:], maskr[:], -BIG)
            nc.vector.tensor_scalar_add(offsc[:], tmpg2[:], BIG)
            nc.vector.tensor_add(out=offsc[:], in0=offsc[:], in1=pos_r[:])
            offsc_i = sb.tile([P, G], I32, tag="offsci")
            nc.vector.tensor_copy(out=offsc_i[:], in_=offsc[:])
            vals = sb.tile([P, G, 3], F32, tag="vals")
            nc.vector.tensor_copy(out=vals[:, :, 0], in_=iotat[:])
            nc.vector.tensor_copy(out=vals[:, :, 1], in_=iotat[:])
            nc.vector.tensor_copy(out=vals[:, :, 2], in_=g_r[:])
            vals2 = vals[:].rearrange('p g f -> p (g f)')
            # scatter the (token, token, gating) triples, OOB-skip unassigned
            for g in range(0, G, SCATTER_BATCH):
                src = vals[:, g, :] if SCATTER_BATCH == 1 else vals2[:, g * 3:(g + SCATTER_BATCH) * 3]
                nc.gpsimd.indirect_dma_start(
                    out=comp[:],
                    out_offset=IndirectOffsetOnAxis(ap=offsc_i[:, g:g + SCATTER_BATCH], axis=0),
                    in_=src, in_offset=None,
                    bounds_check=C - 1, oob_is_err=False,
                )
            # reload compact ids & gatings (padding rows keep host prefill: 0 / N_TOK / 0)
            ids_f = sb.tile([P, CB], F32, tag="idsf")
            nc.scalar.dma_start(out=ids_f[:], in_=bass.AP(comp, 0, [[3, P], [3 * P, CB]]))
            idsc_f = sb.tile([P, CB], F32, tag="idscf")
            nc.sync.dma_start(out=idsc_f[:], in_=bass.AP(comp, 1, [[3, P], [3 * P, CB]]))
            g_load = sb.tile([P, CB], F32, tag="gload")
            nc.sync.dma_start(out=g_load[:], in_=bass.AP(comp, 2, [[3, P], [3 * P, CB]]))
            ids_i = sb.tile([P, CB], I32, tag="idsi")
            nc.vector.tensor_copy(out=ids_i[:], in_=ids_f[:])
            idsc_i = sb.tile([P, CB], I32, tag="idsci")
            nc.vector.tensor_copy(out=idsc_i[:], in_=idsc_f[:])

            # ---- gather x rows (token-major), DMA-transpose to d-major per group ----
            xTg = [bigp.tile([P, D // P, n], BF16, name="xTg_%d" % gi)
                   for gi, (o, n) in enumerate(GROUPS)]
            for c in range(CB):
                gi = next(i for i, (o, n) in enumerate(GROUPS) if c * P < o + n)
                o = GROUPS[gi][0]
                lc = c * P - o   # column offset within group tile
                xg_c = bigp.tile([P, D], BF16, tag="xgc", name="xg_%d" % c, bufs=6)
                nc.gpsimd.indirect_dma_start(
                    out=xg_c[:], out_offset=None,
                    in_=x_bf[:],
                    in_offset=IndirectOffsetOnAxis(ap=ids_i[:, c:c + 1], axis=0),
                )
                for dc in range(D // P):
                    tps = ps.tile([P, P], BF16, space="PSUM", tag="rot", bufs=2,
                                  name="tps_%d_%d" % (c, dc))
                    nc.tensor.transpose(out=tps[:], in_=xg_c[:, dc * P:(dc + 1) * P], identity=identb[:])
                    nc.vector.tensor_copy(out=xTg[gi][:, dc, lc:lc + P], in_=tps[:])


            # ---- PE warmup #3: bridge the scatter/gather window up to mm1 ----
            with tc.tile_wait_until(0.040):
                for i in range(24):
                    nc.tensor.matmul(out=warm_ps[0:E, :], lhsT=wg_sb[:, i % 8, :],
                                     rhs=xT_c[i % XT_CHUNKS][:, i % DCC, :],
                                     start=True, stop=True, skip_group_check=True)

            # ---- mm1: hT[j] = relu(x W1)^2, h-major, per slot group ----
            hT = [bigp.tile([P, H // P, n], BF16, name="hT_%d" % gi)
                  for gi, (o, n) in enumerate(GROUPS)]
            for gi, (o, n) in enumerate(GROUPS):
                for j in range(H // P):
                    hps = mmps.tile([P, 512], F32, space="PSUM", tag="mm",
                                    name="hps_%d_%d" % (gi, j))
                    for dc in range(D // P):
                        nc.tensor.matmul(out=hps[:, :n], lhsT=w1sb[:, dc, j * P:(j + 1) * P],
                                         rhs=xTg[gi][:, dc, :],
                                         start=(dc == 0), stop=(dc == D // P - 1))
                    rl = sb.tile([P, 512], F32, tag="rl", name="rl_%d_%d" % (gi, j), bufs=4)
                    nc.scalar.activation(out=rl[:, :n], in_=hps[:, :n], func=mybir.ActivationFunctionType.Relu)
                    nc.vector.tensor_tensor(out=hT[gi][:, j, :], in0=rl[:, :n], in1=rl[:, :n],
                                            op=mybir.AluOpType.mult)

            # ---- mm2: y = hT^T W2, half-column passes so RS(half0) overlaps pass 1 ----
            for dn in range(2):
                for m in range(CB):
                    gi = next(i for i, (o, n) in enumerate(GROUPS) if m * P < o + n)
                    o = GROUPS[gi][0]
                    lm = m * P - o
                    yrow = sb.tile([P, DH], BF16, tag="yrow", name="yrow_%d_%d" % (dn, m), bufs=12)
                    yps = mmps.tile([P, 512], F32, space="PSUM", tag="mm",
                                    name="yps_%d_%d" % (m, dn))
                    for jj in range(H // P):
                        nc.tensor.matmul(out=yps[:], lhsT=hT[gi][:, jj, lm:lm + P],
                                         rhs=w2sb[:, jj, dn * DH:(dn + 1) * DH],
                                         start=(jj == 0), stop=(jj == H // P - 1))
                    nc.scalar.activation(out=yrow[:], in_=yps[:],
                                         func=mybir.ActivationFunctionType.Copy,
                                         scale=g_load[:, m:m + 1])
                    nc.gpsimd.indirect_dma_start(
                        out=y_half[dn][:],
                        out_offset=IndirectOffsetOnAxis(ap=idsc_i[:, m:m + 1], axis=0),
                        in_=yrow[:], in_offset=None,
                    )
                # combine this half: ReduceScatter(add) overlaps the next pass
                nc.gpsimd.collective_compute(
                    "ReduceScatter", mybir.AluOpType.add,
                    ins=[y_half[dn][0:N_TOK, :]], outs=[y_rs[dn][:]],
                    replica_groups=[list(range(R))],
                )
                eng = nc.sync if dn == 0 else nc.scalar
                eng.dma_start(out=bass.AP(out_shard, dn * DH, [[D, SH], [1, DH]]),
                              in_=bass.AP(y_rs[dn], 0, [[DH, SH], [1, DH]]))

    nc.finalize()
    return nc


# ---------------- host-side constants ----------------
def host_constants():
    ident = np.eye(P, dtype=np.float32)
    lstrict = np.triu(np.ones((P, P), np.float32), k=1)  # [k, m] = 1 iff m > k
    # rows/cols indexed by (g*8 + e) within a 128-slot half (16 g values)
    gg, ee = np.arange(16), np.arange(E)
    gi = np.repeat(gg, E)   # g of row index
    ei = np.tile(ee, 16)    # e of row index
    le00 = ((ei[:, None] == ei[None, :]) & (gi[:, None] < gi[None, :])).astype(np.float32)
    le01 = (ei[:, None] == ei[None, :]).astype(np.float32)
    iota8 = np.broadcast_to(np.arange(E, dtype=np.float32), (P, E)).copy()
    iotat = (np.arange(G, dtype=np.float32)[None, :] * P + np.arange(P, dtype=np.float32)[:, None]).copy()
    return ident, lstrict, le00, le01, iota8, iotat


def build_in_maps(x, Wg, W1, W2):
    x = np.asarray(x); Wg = np.asarray(Wg); W1 = np.asarray(W1); W2 = np.asarray(W2)
    xt = x.reshape(N_TOK, D).astype(np.float32)
    x_bf = xt.astype(ml_dtypes.bfloat16)
    ident, lstrict, le00, le01, iota8, iotat = host_constants()
    in_maps = []
    for r in range(R):
        onehr = np.zeros((P, E), np.float32); onehr[:, r] = 1.0
        in_maps.append({
            "xT_shard": np.ascontiguousarray(xt[r * SH:(r + 1) * SH, :].T),
            "x_bf": x_bf,
            "w1": W1[r].astype(ml_dtypes.bfloat16),
            "w2": W2[r].astype(ml_dtypes.bfloat16),
            "wg": Wg.astype(np.float32),
            "ident": ident, "lstrict": lstrict, "le00": le00, "le01": le01,
            "iota8": iota8, "iotat": iotat, "onehr": onehr,
        })
    return in_maps


_NC_CACHE = {}

def kernel(x, Wg, W1, W2):
    x = np.asarray(x)
    B, T, Dx = x.shape
    in_maps = build_in_maps(x, Wg, W1, W2)
    if "nc" not in _NC_CACHE:
        _NC_CACHE["nc"] = build_kernel()
    from concourse.bass_utils import run_bass_kernel_spmd
    res = run_bass_kernel_spmd(_NC_CACHE["nc"], in_maps, list(range(R)))
    globals()['LAST_RES'] = res
    out = np.concatenate([np.asarray(res.results[r]["out_shard"]).astype(np.float32)
                          for r in range(R)], axis=0)
    return out.reshape(B, T, Dx)


if __name__ == "__main__":
    d = np.load("/tmp/inputs.npz")
    out = kernel(d["x"], d["Wg"], d["W1"], d["W2"])
    ref = np.load("/tmp/ref_out.npy")
    err = np.abs(out - ref).max() / np.abs(ref).max()
    print("rel err (absmax):", err)
